# revision 43
# baseline (speedup 1.0000x reference)
"""Trainium2 Bass kernel for nn_EquivariantInteractionBlock.

Strategy (edge/graph parallel, 8 cores):
- Host: sort edges by dst; split into 8 node-aligned contiguous ranges with
  ~E/8 edges each. Per core, pack edges into supertiles: <=1024 edges
  covering a window of <=128 consecutive dst nodes. Host gathers raw x rows
  by edge_src, precomputes the cosine cutoff, builds one-hot scatter
  matrices, and pre-swizzles everything into DMA-friendly bf16 layouts.
- Device per supertile (all matmuls bf16, fp32 PSUM accumulate):
  * radial MLP hidden: h = silu(rbf@W1) via one matmul + one silu per
    512-edge group (msg+gate hidden together, feature-major)
  * per-edge TP weights + gate logit: per 128-edge subtile one stationary
    load (h slice) and two matmuls streaming 512+66 weight columns
  * sigmoid via tanh (same ACT table set as silu -> no table reloads),
    rsqrt for RMS norms via DVE Newton iteration
  * tensor-product products on VectorE, i-reductions for paths 1/2 ride
    the scatter matmul as extra columns
  * scatter-add via host-built one-hot selection matrices (bf16 matmul)
  * node phase: normalize, two packed PE transposes, small accumulating
    matmuls for msg/update/self linears, residual in fp32
- Each core owns a disjoint node range: no collectives; host concatenates
  per-core output rows.
"""

import math
import numpy as np
import ml_dtypes

import concourse.bass as bass
import concourse.mybir as mybir
import concourse.tile as tile
from concourse.bass_utils import run_bass_kernel_spmd
from concourse.masks import make_identity

F32 = mybir.dt.float32
BF16 = mybir.dt.bfloat16
AF = mybir.ActivationFunctionType
OP = mybir.AluOpType
BF = ml_dtypes.bfloat16

N = 50000
E = 400000
MUL0 = 16
MUL1 = 8
RBF = 8
HID = 64
CUTOFF = 5.0
EPS = 1e-8
INV3 = float(1.0 / np.sqrt(np.float32(3.0)))
APATH = float(1.0 / math.sqrt(MUL0 + MUL1))
NCORE = 8
SUB = 128          # edges per subtile
SPS = 8            # subtiles per supertile
SUPE = SUB * SPS   # 1024 edges per supertile
NPW = 128          # node window per supertile

# P (product/scatter) column layout
C_P12 = 0           # 384: (j16 x [i16 p1 | i8 p2]) unreduced
C_M13 = 384         # 24: m1 path3 (c3,j8) reduced
C_M14 = 408         # 24: m1 path4 (c3,j8) reduced
C_EW = 432          # 1: edge weight (norm channel)
PCOLS = 433

# irrep-norm scale folding: device computes rsqrt(sum of squares); the
# 1/sqrt(mean) = sqrt(16) (s) / sqrt(8) (v) factors are folded into weights
FS = 4.0
FV = float(np.sqrt(8.0))


def _host_prep(x, edge_src, edge_dst, edge_sh, edge_rbf, edge_len,
               w_r1, b_r1, w_r2, b_r2, w_g1, b_g1, w_g2, b_g2,
               Wm_s, Wm_v, Wu_s, Wu_v, Ws_s, Ws_v, res_scale):
    order = np.argsort(edge_dst, kind="stable")
    src_s = edge_src[order]
    dst_s = edge_dst[order]
    sh_s = edge_sh[order]
    rbf_s = edge_rbf[order]
    len_s = edge_len[order]

    deg = np.bincount(edge_dst, minlength=N).astype(np.int64)
    cum = np.concatenate([[0], np.cumsum(deg)])

    bounds = [0]
    for k in range(1, NCORE):
        bounds.append(int(np.searchsorted(cum, k * E // NCORE)))
    bounds.append(N)

    cores = []
    for k in range(NCORE):
        n0, n1 = bounds[k], bounds[k + 1]
        sups = []  # (node_base, estart, ecnt)
        nb = n0
        while nb < n1:
            nn = nb
            cnt = 0
            while nn < n1 and nn - nb < NPW and cnt + deg[nn] <= SUPE:
                cnt += int(deg[nn])
                nn += 1
            sups.append((nb, int(cum[nb]), cnt))
            nb = nn
        cores.append((n0, n1, sups))

    nsup = max(len(c[2]) for c in cores)

    # ---- host-transformed weights (shared across cores) ----
    w1p = np.zeros((9, 128), np.float32)
    w1p[:8, :64] = w_r1
    w1p[:8, 64:] = w_g1
    w1p[8, :64] = b_r1
    w1p[8, 64:] = b_g1

    # w2e [128, 578]: rows 0:64 = w_r2 (reordered cols), rows 64:128 zero
    # except gate col. cols: 0:384 interleaved (j16 x [i16 p1 | i8 p2]),
    # 384:512 p3 (j8,i16), 512:576 p4 (j8,i8), 576 gate, 577 pad
    # block scales fold the 1/sqrt(mean)-vs-rsqrt(sum) factors: paths
    # contracting normalized s get FS, normalized v get FV
    w2e = np.zeros((128, 578), np.float32)
    wsrc = w_r2.astype(np.float32)  # [64, 576]
    # p1: our col j*24+i <- ref col i*16+j (i16, j16)
    jj, ii = np.meshgrid(np.arange(16), np.arange(16), indexing="ij")
    w2e[:64, (jj * 24 + ii).ravel()] = FS * wsrc[:, (ii * 16 + jj).ravel()]
    # p2: our col j*24+16+i <- ref col 256+i*16+j (i8, j16)
    jj, ii = np.meshgrid(np.arange(16), np.arange(8), indexing="ij")
    w2e[:64, (jj * 24 + 16 + ii).ravel()] = FV * wsrc[:, (256 + ii * 16 + jj).ravel()]
    # p3: our col 384+j*16+i <- ref col 384+i*8+j (i16, j8)
    jj, ii = np.meshgrid(np.arange(8), np.arange(16), indexing="ij")
    w2e[:64, (384 + jj * 16 + ii).ravel()] = FS * wsrc[:, (384 + ii * 8 + jj).ravel()]
    # p4: our col 512+j*8+i <- ref col 512+i*8+j (i8, j8)
    jj, ii = np.meshgrid(np.arange(8), np.arange(8), indexing="ij")
    w2e[:64, (512 + jj * 8 + ii).ravel()] = FV * wsrc[:, (512 + ii * 8 + jj).ravel()]
    w2e[64:128, 576] = w_g2[:, 0]

    # b_r2 row, same column order and scales (only used when b_r2 != 0)
    br2e = np.zeros((1, 578), np.float32)
    bsrc = b_r2.astype(np.float32)
    jj, ii = np.meshgrid(np.arange(16), np.arange(16), indexing="ij")
    br2e[0, (jj * 24 + ii).ravel()] = FS * bsrc[(ii * 16 + jj).ravel()]
    jj, ii = np.meshgrid(np.arange(16), np.arange(8), indexing="ij")
    br2e[0, (jj * 24 + 16 + ii).ravel()] = FV * bsrc[(256 + ii * 16 + jj).ravel()]
    jj, ii = np.meshgrid(np.arange(8), np.arange(16), indexing="ij")
    br2e[0, (384 + jj * 16 + ii).ravel()] = FS * bsrc[(384 + ii * 8 + jj).ravel()]
    jj, ii = np.meshgrid(np.arange(8), np.arange(8), indexing="ij")
    br2e[0, (512 + jj * 8 + ii).ravel()] = FV * bsrc[(512 + ii * 8 + jj).ravel()]
    use_bias = bool(np.any(b_r2 != 0.0))

    s0 = 1.0 / math.sqrt(MUL0)
    s1 = 1.0 / math.sqrt(MUL1)
    wms = (Wm_s * s0).astype(np.float32)                      # [16,24]
    wmv = np.zeros((24, 24), np.float32)
    wuv = np.zeros((24, 24), np.float32)
    wsv = np.zeros((24, 24), np.float32)
    for c in range(3):
        for j in range(8):
            for j2 in range(8):
                wmv[c * 8 + j, c * 8 + j2] = Wm_v[j, j2] * s1
                wuv[c * 8 + j, j2 * 3 + c] = Wu_v[j, j2] * s1
                wsv[j * 3 + c, j2 * 3 + c] = Ws_v[j, j2] * s1 * FV
    wus = (Wu_s * s0).astype(np.float32)
    wss = (Ws_s * s0 * FS).astype(np.float32)
    rep = np.zeros((8, 24), np.float32)
    for c in range(3):
        for j in range(8):
            rep[j, c * 8 + j] = 1.0

    shared = dict(
        w1p=w1p.astype(BF), w2e=w2e.astype(BF), br2e=br2e.astype(BF),
        wms=wms.astype(BF), wmv=wmv.astype(BF), rep=rep.astype(BF),
        wus=wus.astype(BF), wss=wss.astype(BF),
        wuv=wuv.astype(BF), wsv=wsv.astype(BF))

    in_maps = []
    metas = []
    for k in range(NCORE):
        n0, n1, sups = cores[k]
        ns = len(sups)
        idx = np.full((nsup, SUPE), -1, np.int64)
        base_arr = np.full((nsup,), n1, np.int64)
        span_arr = np.zeros((nsup,), np.int64)
        for si, (nb, es, cnt) in enumerate(sups):
            idx[si, :cnt] = np.arange(es, es + cnt)
            base_arr[si] = nb
            span_arr[si] = min(NPW, n1 - nb)
        mask = idx >= 0
        ic = np.clip(idx, 0, E - 1)

        feat = x[src_s[ic]]                                    # [nsup,SUPE,40]
        shp = sh_s[ic].astype(np.float32)
        lenp = len_s[ic].astype(np.float32)
        cw = 0.5 * (np.cos(np.pi * lenp / CUTOFF) + 1.0) * (lenp < CUTOFF)
        cwh = np.where(mask, 0.5 * cw, 0.0).astype(np.float32)  # [nsup,SUPE]
        rbfp = np.where(mask[..., None], rbf_s[ic], 0.0).astype(np.float32)
        dstl = np.where(mask, dst_s[ic] - base_arr[:, None], 0).astype(np.int64)

        # scal cols: sh0*APATH, sh1*APATH (3), sh1*APATH*INV3 (3), cwh
        scal = np.concatenate(
            [APATH * shp[..., 0:1], APATH * shp[..., 1:4],
             (APATH * INV3) * shp[..., 1:4], cwh[..., None]],
            axis=-1).astype(np.float32)                         # [nsup,SUPE,8]

        # swizzle [nsup, SUPE, F] -> [nsup, 128, SPS, F]
        def sw(a, dt):
            f = a.shape[-1]
            return np.ascontiguousarray(
                a.reshape(nsup, SPS, SUB, f).transpose(0, 2, 1, 3)).astype(dt)

        rbft = np.concatenate(
            [rbfp.reshape(nsup * 2, 512, 8).transpose(0, 2, 1),
             np.ones((nsup * 2, 1, 512), np.float32)], axis=1)  # [2nsup,9,512]

        # one-hot scatter matrices [nsup, SPS, SUB, NPW] -> [nsup,128,SPS*128]
        sel = np.zeros((nsup, SPS, SUB, NPW), np.float32)
        si_i, e_i = np.nonzero(mask)
        t_i = e_i // SUB
        p_i = e_i % SUB
        sel[si_i, t_i, p_i, dstl[si_i, e_i]] = 1.0
        sel = np.ascontiguousarray(
            sel.transpose(0, 2, 1, 3).reshape(nsup, SUB, SPS * NPW)).astype(BF)

        nodes = np.clip(base_arr[:, None] + np.arange(NPW)[None, :], 0, N - 1)
        xown = x[nodes].astype(np.float32)                      # [nsup,128,40]

        m = dict(shared)
        m.update(feat=sw(feat, BF), scal=sw(scal, np.float32),
                 rbft=np.ascontiguousarray(rbft).astype(BF), sel=sel,
                 xown=np.ascontiguousarray(xown))
        in_maps.append(m)
        metas.append((n0, n1, base_arr, span_arr, ns))

    return in_maps, metas, nsup, float(b_g2[0]), float(res_scale), use_bias


def _newton_rsqrt(nc, y, r, rh, w, msq):
    """y = 1/sqrt(msq) (all args APs of equal shape; r/rh/w scratch).
    msq is a sum of >=1 squared N(0,1) draws (roughly [1, 64]);
    r = 1/msq in ~[0.015, 1]; y = sqrt(r) by Heron from y0 = r + 0.25."""
    nc.vector.reciprocal(out=r, in_=msq)
    nc.vector.tensor_scalar_mul(out=rh, in0=r, scalar1=0.5)
    nc.vector.tensor_scalar(out=y, in0=r, scalar1=0.25, scalar2=None,
                            op0=OP.add)
    for _ in range(2):
        nc.vector.reciprocal(out=w, in_=y)
        nc.vector.tensor_tensor(out=w, in0=w, in1=rh, op=OP.mult)
        nc.vector.scalar_tensor_tensor(out=y, in0=y, scalar=0.5,
                                       in1=w, op0=OP.mult, op1=OP.add)


def build_program(nsup, bg2, res, use_bias):
    import concourse.bacc as bacc
    nc = bacc.Bacc("TRN2", target_bir_lowering=False, debug=False,
                   num_devices=NCORE)

    feat_d = nc.dram_tensor("feat", [nsup, 128, SPS, 40], BF16, kind="ExternalInput")
    scal_d = nc.dram_tensor("scal", [nsup, 128, SPS, 8], F32, kind="ExternalInput")
    rbft_d = nc.dram_tensor("rbft", [nsup * 2, 9, 512], BF16, kind="ExternalInput")
    sel_d = nc.dram_tensor("sel", [nsup, 128, SPS * 128], BF16, kind="ExternalInput")
    xown_d = nc.dram_tensor("xown", [nsup, 128, 40], F32, kind="ExternalInput")
    w1p_d = nc.dram_tensor("w1p", [9, 128], BF16, kind="ExternalInput")
    w2e_d = nc.dram_tensor("w2e", [128, 578], BF16, kind="ExternalInput")
    br2e_d = nc.dram_tensor("br2e", [1, 578], BF16, kind="ExternalInput")
    wms_d = nc.dram_tensor("wms", [16, 24], BF16, kind="ExternalInput")
    wmv_d = nc.dram_tensor("wmv", [24, 24], BF16, kind="ExternalInput")
    rep_d = nc.dram_tensor("rep", [8, 24], BF16, kind="ExternalInput")
    wus_d = nc.dram_tensor("wus", [16, 16], BF16, kind="ExternalInput")
    wss_d = nc.dram_tensor("wss", [16, 16], BF16, kind="ExternalInput")
    wuv_d = nc.dram_tensor("wuv", [24, 24], BF16, kind="ExternalInput")
    wsv_d = nc.dram_tensor("wsv", [24, 24], BF16, kind="ExternalInput")
    out_d = nc.dram_tensor("out", [nsup, 128, 40], F32, kind="ExternalOutput")

    with tile.TileContext(nc) as tc:
        with (
            tc.tile_pool(name="const", bufs=1) as cp,
            tc.tile_pool(name="io", bufs=4) as iop,
            tc.tile_pool(name="mid", bufs=3) as mp,
            tc.tile_pool(name="pp", bufs=3) as ppp,
            tc.tile_pool(name="nd", bufs=3) as ndp,
            tc.tile_pool(name="psh", bufs=2, space="PSUM") as psH,
            tc.tile_pool(name="psw0", bufs=2, space="PSUM") as psW0,
            tc.tile_pool(name="psa", bufs=2, space="PSUM") as psA,
        ):
            w1p = cp.tile([9, 128], BF16, tag="w1p")
            w2e = cp.tile([128, 578], BF16, tag="w2e")
            br2e = cp.tile([1, 578], BF16, tag="br2e")
            wms = cp.tile([16, 24], BF16, tag="wms")
            wmv = cp.tile([24, 24], BF16, tag="wmv")
            rep = cp.tile([8, 24], BF16, tag="rep")
            wus = cp.tile([16, 16], BF16, tag="wus")
            wss = cp.tile([16, 16], BF16, tag="wss")
            wuv = cp.tile([24, 24], BF16, tag="wuv")
            wsv = cp.tile([24, 24], BF16, tag="wsv")
            ident = cp.tile([128, 128], F32, tag="ident")
            for t, d in [(w1p, w1p_d), (w2e, w2e_d), (br2e, br2e_d),
                         (wms, wms_d), (wmv, wmv_d), (rep, rep_d),
                         (wus, wus_d), (wss, wss_d), (wuv, wuv_d),
                         (wsv, wsv_d)]:
                nc.sync.dma_start(out=t[:], in_=d[:])
            make_identity(nc, ident[:])
            cbg2h = cp.tile([128, 1], F32, tag="cbg2h")
            nc.gpsimd.memset(cbg2h[:], 0.5 * bg2)
            onesr = cp.tile([1, 128], BF16, tag="onesr")
            nc.gpsimd.memset(onesr[:], 1.0)

            def tposed(src_ap, rows, tag):
                tp = psH.tile([rows, 128], F32, tag="h")
                dst = ndp.tile([rows, 128], BF16, tag=tag)
                nc.tensor.transpose(out=tp[:], in_=src_ap, identity=ident[:])
                nc.scalar.copy(out=dst[:], in_=tp[:])
                return dst

            def emit_node(s_idx, agg, xo, inv):
                m0 = ndp.tile([128, 16], F32, tag="m0")
                nc.vector.reduce_sum(
                    out=m0[:],
                    in_=agg[:, 0:384].rearrange("p (j i) -> p j i", i=24),
                    axis=mybir.AxisListType.X)
                v1 = ndp.tile([128, 24], F32, tag="v1")
                nc.vector.reduce_sum(
                    out=v1[:],
                    in_=agg[:, C_M13:C_M13 + 48].rearrange(
                        "p (a b) -> p b a", b=24),
                    axis=mybir.AxisListType.X)
                nrm = ndp.tile([128, 1], F32, tag="nrm")
                nc.vector.tensor_scalar_max(out=nrm[:], in0=agg[:, C_EW, None],
                                            scalar1=EPS)
                rinv = ndp.tile([128, 1], F32, tag="rinv")
                nc.vector.reciprocal(out=rinv[:], in_=nrm[:])

                cat_s = ndp.tile([128, 32], F32, tag="cat_s")
                cat_v = ndp.tile([128, 48], F32, tag="cat_v")
                nc.gpsimd.tensor_tensor(
                    out=cat_s[:, 0:16], in0=m0[:],
                    in1=rinv[:].to_broadcast([128, 16]), op=OP.mult)
                nc.gpsimd.tensor_tensor(
                    out=cat_v[:, 0:24], in0=v1[:],
                    in1=rinv[:].to_broadcast([128, 24]), op=OP.mult)
                nc.gpsimd.tensor_tensor(
                    out=cat_s[:, 16:32], in0=xo[:, 0:16],
                    in1=inv[:, 2, 0:1].to_broadcast([128, 16]), op=OP.mult)
                nc.gpsimd.tensor_tensor(
                    out=cat_v[:, 24:48], in0=xo[:, 16:40],
                    in1=inv[:, 2, 1:2].to_broadcast([128, 24]), op=OP.mult)

                aggT_s = tposed(cat_s[:, 0:16], 16, "aTs")
                xnT_s = tposed(cat_s[:, 16:32], 16, "xnTs")
                aggT_v = tposed(cat_v[:, 0:24], 24, "aTv")
                xnT_v = tposed(cat_v[:, 24:48], 24, "xnTv")

                scp = psH.tile([16, 128], F32, tag="h")
                nc.tensor.matmul(out=scp[:], lhsT=wms[:, 0:16], rhs=aggT_s[:],
                                 start=True, stop=True)
                scalT = ndp.tile([16, 128], BF16, tag="scalT")
                nc.scalar.activation(out=scalT[:], in_=scp[:], func=AF.Silu)
                gcp = psH.tile([8, 128], F32, tag="h")
                nc.tensor.matmul(out=gcp[:], lhsT=wms[:, 16:24], rhs=aggT_s[:],
                                 start=True, stop=True)
                gtt = ndp.tile([8, 128], BF16, tag="gtt")
                nc.scalar.activation(out=gtt[:], in_=gcp[:], func=AF.Tanh,
                                     scale=0.5)
                gT = ndp.tile([8, 128], BF16, tag="gT")
                nc.scalar.activation(out=gT[:], in_=gtt[:], func=AF.Copy,
                                     scale=0.5, bias=0.5)

                vvp = psH.tile([24, 128], F32, tag="h")
                nc.tensor.matmul(out=vvp[:], lhsT=wmv[:], rhs=aggT_v[:],
                                 start=True, stop=True)
                grp = psH.tile([24, 128], F32, tag="h")
                nc.tensor.matmul(out=grp[:], lhsT=rep[:], rhs=gT[:],
                                 start=True, stop=True)
                vvc = ndp.tile([24, 128], BF16, tag="vvc")
                nc.scalar.copy(out=vvc[:], in_=vvp[:])
                vgT = ndp.tile([24, 128], BF16, tag="vgT")
                nc.vector.tensor_tensor(out=vgT[:], in0=vvc[:], in1=grp[:],
                                        op=OP.mult)

                osp = psH.tile([16, 128], F32, tag="h")
                nc.tensor.matmul(out=osp[:], lhsT=wus[:], rhs=scalT[:],
                                 start=True, stop=False)
                nc.tensor.matmul(out=osp[:], lhsT=wss[:], rhs=xnT_s[:],
                                 start=False, stop=True)
                ovp = psH.tile([24, 128], F32, tag="h")
                nc.tensor.matmul(out=ovp[:], lhsT=wuv[:], rhs=vgT[:],
                                 start=True, stop=False)
                nc.tensor.matmul(out=ovp[:], lhsT=wsv[:], rhs=xnT_v[:],
                                 start=False, stop=True)

                fTs = ndp.tile([16, 128], F32, tag="fTs")
                nc.scalar.activation(out=fTs[:], in_=osp[:], func=AF.Copy,
                                     scale=res)
                fTv = ndp.tile([24, 128], F32, tag="fTv")
                nc.scalar.activation(out=fTv[:], in_=ovp[:], func=AF.Copy,
                                     scale=res)
                fps = psH.tile([128, 16], F32, tag="h")
                nc.tensor.transpose(out=fps[:], in_=fTs[:],
                                    identity=ident[0:16, 0:16])
                fpv = psH.tile([128, 24], F32, tag="h")
                nc.tensor.transpose(out=fpv[:], in_=fTv[:],
                                    identity=ident[0:24, 0:24])
                outt = ndp.tile([128, 40], F32, tag="outt")
                nc.vector.tensor_tensor(out=outt[:, 0:16], in0=xo[:, 0:16],
                                        in1=fps[:], op=OP.add)
                nc.vector.tensor_tensor(out=outt[:, 16:40], in0=xo[:, 16:40],
                                        in1=fpv[:], op=OP.add)
                nc.sync.dma_start(out=out_d[s_idx], in_=outt[:])

            pend = None
            for s in range(nsup):
                feats = iop.tile([128, SPS, 40], BF16, tag="feat")
                scals = iop.tile([128, SPS, 8], F32, tag="scal")
                selt = iop.tile([128, SPS, 128], BF16, tag="sel")
                xo = iop.tile([128, 40], F32, tag="xo")
                nc.sync.dma_start(out=feats[:], in_=feat_d[s])
                nc.sync.dma_start(out=scals[:], in_=scal_d[s])
                nc.sync.dma_start(out=selt[:], in_=sel_d[s])
                nc.sync.dma_start(out=xo[:], in_=xown_d[s])

                # ---- joint RMS factors (raw sums of squares; mean-scales are
                # folded into w2e/wss/wsv on host) ----
                sq = mp.tile([128, SPS, 40], F32, tag="sq")
                nc.gpsimd.tensor_tensor(out=sq[:], in0=feats[:], in1=feats[:],
                                        op=OP.mult)
                xsq = mp.tile([128, 40], F32, tag="xsq")
                nc.gpsimd.tensor_tensor(out=xsq[:], in0=xo[:], in1=xo[:],
                                        op=OP.mult)
                # rows: 0 edge-s, 1 edge-v, 2 node ([s, v] in cols 0:2)
                ms = mp.tile([128, 3, SPS], F32, tag="ms")
                nc.vector.memset(ms[:, 2, 2:SPS], 1.0)
                nc.vector.reduce_sum(out=ms[:, 0, :], in_=sq[:, :, 0:16],
                                     axis=mybir.AxisListType.X)
                nc.vector.reduce_sum(out=ms[:, 1, :], in_=sq[:, :, 16:40],
                                     axis=mybir.AxisListType.X)
                nc.vector.reduce_sum(out=ms[:, 2, 0:1], in_=xsq[:, None, 0:16],
                                     axis=mybir.AxisListType.X)
                nc.vector.reduce_sum(out=ms[:, 2, 1:2], in_=xsq[:, None, 16:40],
                                     axis=mybir.AxisListType.X)
                inv = mp.tile([128, 3, SPS], F32, tag="inv")
                nr = mp.tile([128, 3, SPS], F32, tag="nr")
                nrh = mp.tile([128, 3, SPS], F32, tag="nrh")
                nw = mp.tile([128, 3, SPS], F32, tag="nw")
                _newton_rsqrt(nc, inv[:], nr[:], nrh[:], nw[:], ms[:])
                # inv rows: 0 = edge-s, 1 = edge-v, 2 = [node-s, node-v, ...]

                st = mp.tile([128, SPS, 16], BF16, tag="st")
                vt = mp.tile([128, SPS, 24], BF16, tag="vt")
                nc.gpsimd.tensor_tensor(
                    out=st[:], in0=feats[:, :, 0:16],
                    in1=inv[:, 0, :, None].to_broadcast([128, SPS, 16]),
                    op=OP.mult)
                nc.gpsimd.tensor_tensor(
                    out=vt[:], in0=feats[:, :, 16:40],
                    in1=inv[:, 1, :, None].to_broadcast([128, SPS, 24]),
                    op=OP.mult)

                # ---- radial MLP hidden for both groups ----
                hsil = []
                for g in range(2):
                    rbft = iop.tile([9, 512], BF16, tag="rbft")
                    nc.sync.dma_start(out=rbft[:], in_=rbft_d[s * 2 + g])
                    hp = psH.tile([128, 512], F32, tag="h")
                    nc.tensor.matmul(out=hp[:], lhsT=w1p[:], rhs=rbft[:],
                                     start=True, stop=True)
                    hs = mp.tile([128, 512], BF16, tag=f"hs{g}")
                    nc.scalar.activation(out=hs[:], in_=hp[:], func=AF.Silu)
                    hsil.append(hs)

                # supertile-wide chain/product tiles
                gw8 = mp.tile([128, SPS], F32, tag="gw8")
                o4 = mp.tile([128, SPS], BF16, tag="o4")
                o3cs = mp.tile([128, SPS, 6], BF16, tag="o3cs")
                i4 = mp.tile([128, SPS], BF16, tag="i4")
                g12 = mp.tile([128, SPS, 24], BF16, tag="g12")
                g4 = mp.tile([128, SPS, 24], BF16, tag="g4")
                u3 = mp.tile([128, SPS, 8], BF16, tag="u3")
                a2 = ppp.tile([128, SPS, 8, 3], BF16, tag="a2")
                t3d = ppp.tile([128, SPS, 8, 16], BF16, tag="t3")
                t4d = ppp.tile([128, SPS, 3, 64], BF16, tag="t4")
                P = ppp.tile([128, SPS, PCOLS], BF16, tag="P")

                agg = psA.tile([128, PCOLS], F32, tag="agg")
                for g in range(2):
                    sl4 = slice(g * 4, g * 4 + 4)
                    # gate+p4 matmuls first so the scalar chain overlaps the
                    # big weight matmuls that follow
                    pw1 = psH.tile([128, 4, 66], F32, tag="h")
                    for tl in range(4):
                        lhs = hsil[g][:, tl * 128:(tl + 1) * 128]
                        if use_bias:
                            nc.tensor.matmul(out=pw1[:, tl, :], lhsT=onesr[:],
                                             rhs=br2e[:, 512:578],
                                             start=True, stop=False)
                        nc.tensor.matmul(out=pw1[:, tl, :], lhsT=lhs,
                                         rhs=w2e[:, 512:578],
                                         start=not use_bias, stop=True)

                    # ---- per-edge scalar chain (gpsimd; group batch) ----
                    nc.scalar.activation(out=gw8[:, sl4], in_=pw1[:, :, 64],
                                         func=AF.Tanh, scale=0.5, bias=cbg2h[:])
                    # ew = (tanh+1)*cwh, written straight into P's norm col
                    nc.vector.scalar_tensor_tensor(
                        out=P[:, sl4, C_EW], in0=gw8[:, sl4], scalar=1.0,
                        in1=scals[:, sl4, 7], op0=OP.add, op1=OP.mult)
                    ew = P[:, sl4, C_EW]
                    nc.gpsimd.tensor_tensor(out=o4[:, sl4], in0=ew,
                                            in1=scals[:, sl4, 0], op=OP.mult)
                    nc.gpsimd.tensor_tensor(
                        out=o3cs[:, sl4, :], in0=scals[:, sl4, 1:7],
                        in1=ew[:, :, None].to_broadcast([128, 4, 6]), op=OP.mult)
                    nc.gpsimd.tensor_tensor(out=i4[:, sl4], in0=o4[:, sl4],
                                            in1=inv[:, 0, sl4], op=OP.mult)
                    nc.gpsimd.tensor_tensor(
                        out=g12[:, sl4, 0:16], in0=feats[:, sl4, 0:16],
                        in1=i4[:, sl4, None].to_broadcast([128, 4, 16]),
                        op=OP.mult)
                    nc.gpsimd.tensor_tensor(
                        out=g4[:, sl4, :], in0=vt[:, sl4, :],
                        in1=o4[:, sl4, None].to_broadcast([128, 4, 24]),
                        op=OP.mult)
                    nc.gpsimd.tensor_tensor(
                        out=a2[:, sl4],
                        in0=vt[:, sl4, :].rearrange("p s (i c) -> p s i c", c=3),
                        in1=o3cs[:, sl4, None, 3:6].to_broadcast([128, 4, 8, 3]),
                        op=OP.mult)
                    with nc.allow_low_precision(reason="3-term bf16 sum"):
                        nc.vector.reduce_sum(out=g12[:, sl4, 16:24],
                                             in_=a2[:, sl4],
                                             axis=mybir.AxisListType.X)

                    # ---- weight matmuls in double-buffered pairs, with the
                    # products for each pair issued as soon as it lands ----
                    for k in range(2):
                        sl2 = slice(g * 4 + k * 2, g * 4 + k * 2 + 2)
                        pw0 = psW0.tile([128, 2, 512], F32, tag="pw0")
                        for tl2 in range(2):
                            tl = k * 2 + tl2
                            lhs = hsil[g][:, tl * 128:(tl + 1) * 128]
                            if use_bias:
                                nc.tensor.matmul(out=pw0[:, tl2, :],
                                                 lhsT=onesr[:],
                                                 rhs=br2e[:, 0:512],
                                                 start=True, stop=False)
                            nc.tensor.matmul(out=pw0[:, tl2, :], lhsT=lhs,
                                             rhs=w2e[:, 0:512],
                                             start=not use_bias, stop=True)
                        # stage the pair to SBUF bf16 (ACT) so the products
                        # run in the fast all-SBUF 16-bit DVE mode
                        pwS = mp.tile([128, 2, 512], BF16, tag="pwS")
                        nc.scalar.copy(out=pwS[:], in_=pw0[:])
                        nc.vector.tensor_tensor(
                            out=P[:, sl2, 0:384].rearrange(
                                "p s (j i) -> p s j i", i=24),
                            in0=pwS[:, :, 0:384].rearrange(
                                "p s (j i) -> p s j i", i=24),
                            in1=g12[:, sl2, None, :].to_broadcast(
                                [128, 2, 16, 24]),
                            op=OP.mult)
                        nc.vector.tensor_tensor(
                            out=t3d[:, sl2],
                            in0=pwS[:, :, 384:512].rearrange(
                                "p s (j i) -> p s j i", i=16),
                            in1=st[:, sl2, None, :].to_broadcast(
                                [128, 2, 8, 16]),
                            op=OP.mult)
                    with nc.allow_low_precision(reason="16-term bf16 sum"):
                        nc.vector.reduce_sum(out=u3[:, sl4], in_=t3d[:, sl4],
                                             axis=mybir.AxisListType.X)
                    # stage p4 block to SBUF (ACT) so t4 runs in fast DVE mode
                    p4s = mp.tile([128, 4, 64], BF16, tag="p4s")
                    nc.scalar.copy(out=p4s[:], in_=pw1[:, :, 0:64])
                    g4r = g4[:, sl4, :].rearrange("p s (i c) -> p s i c", c=3)
                    for c in range(3):
                        nc.gpsimd.tensor_tensor(
                            out=t4d[:, sl4, c, :].rearrange(
                                "p s (j i) -> p s j i", i=8),
                            in0=p4s[:].rearrange("p s (j i) -> p s j i", i=8),
                            in1=g4r[:, :, :, c][:, :, None, :].to_broadcast(
                                [128, 4, 8, 8]),
                            op=OP.mult)
                    nc.gpsimd.tensor_tensor(
                        out=P[:, sl4, C_M13:C_M13 + 24].rearrange(
                            "p s (c j) -> p s c j", j=8),
                        in0=u3[:, sl4, None, :].to_broadcast([128, 4, 3, 8]),
                        in1=o3cs[:, sl4, 0:3, None].to_broadcast([128, 4, 3, 8]),
                        op=OP.mult)
                    with nc.allow_low_precision(reason="8-term bf16 sum"):
                        nc.vector.reduce_sum(
                            out=P[:, sl4, C_M14:C_M14 + 24],
                            in_=t4d[:, sl4].rearrange(
                                "p s c (j i) -> p s (c j) i", i=8),
                            axis=mybir.AxisListType.X)
                    # scatter this group's subtiles while the next group's
                    # matmuls proceed
                    for tl in range(4):
                        t = g * 4 + tl
                        nc.tensor.matmul(out=agg[:], lhsT=selt[:, t, :],
                                         rhs=P[:, t, :],
                                         start=(t == 0), stop=(t == SPS - 1))

                # node phase is software-pipelined by one supertile: emit the
                # PREVIOUS supertile's node phase here so it fills engine gaps
                # while this supertile's edge phase runs
                if pend is not None:
                    emit_node(*pend)
                pend = (s, agg, xo, inv)
            emit_node(*pend)

    nc.compile()
    return nc


_CACHE = {}


def kernel(**inputs):
    in_maps, metas, nsup, bg2, res, use_bias = _host_prep(**inputs)
    key = (nsup, bg2, res, use_bias)
    if key not in _CACHE:
        _CACHE[key] = build_program(nsup, bg2, res, use_bias)
    nc = _CACHE[key]
    r = run_bass_kernel_spmd(nc, in_maps, list(range(NCORE)))
    out = np.zeros((N, 40), np.float32)
    for k in range(NCORE):
        n0, n1, base_arr, span_arr, ns = metas[k]
        ob = r.results[k]["out"]
        for si in range(ns):
            sp = int(span_arr[si])
            if sp > 0:
                b = int(base_arr[si])
                out[b:b + sp] = ob[si, :sp]
    return out


# revision 44
# speedup vs baseline: 1.1931x; 1.1931x over previous
"""Trainium2 Bass kernel for nn_EquivariantInteractionBlock.

Strategy (edge/graph parallel, 8 cores):
- Host: sort edges by dst; split into 8 node-aligned contiguous ranges with
  ~E/8 edges each. Per core, pack edges into supertiles: <=1024 edges
  covering a window of <=128 consecutive dst nodes. Host gathers raw x rows
  by edge_src, precomputes the cosine cutoff, builds one-hot scatter
  matrices, and pre-swizzles everything into DMA-friendly bf16 layouts.
- Device per supertile (all matmuls bf16, fp32 PSUM accumulate):
  * radial MLP hidden: h = silu(rbf@W1) via one matmul + one silu per
    512-edge group (msg+gate hidden together, feature-major)
  * per-edge TP weights + gate logit: per 128-edge subtile one stationary
    load (h slice) and two matmuls streaming 512+66 weight columns
  * sigmoid via tanh (same ACT table set as silu -> no table reloads),
    rsqrt for RMS norms via DVE Newton iteration
  * tensor-product products on VectorE, i-reductions for paths 1/2 ride
    the scatter matmul as extra columns
  * scatter-add via host-built one-hot selection matrices (bf16 matmul)
  * node phase: normalize, two packed PE transposes, small accumulating
    matmuls for msg/update/self linears, residual in fp32
- Each core owns a disjoint node range: no collectives; host concatenates
  per-core output rows.
"""

import math
import numpy as np
import ml_dtypes

import concourse.bass as bass
import concourse.mybir as mybir
import concourse.tile as tile
from concourse.bass_utils import run_bass_kernel_spmd
from concourse.masks import make_identity

F32 = mybir.dt.float32
BF16 = mybir.dt.bfloat16
AF = mybir.ActivationFunctionType
OP = mybir.AluOpType
BF = ml_dtypes.bfloat16

N = 50000
E = 400000
MUL0 = 16
MUL1 = 8
RBF = 8
HID = 64
CUTOFF = 5.0
EPS = 1e-8
INV3 = float(1.0 / np.sqrt(np.float32(3.0)))
APATH = float(1.0 / math.sqrt(MUL0 + MUL1))
NCORE = 8
SUB = 128          # edges per subtile
SPS = 8            # subtiles per supertile
SUPE = SUB * SPS   # 1024 edges per supertile
NPW = 128          # node window per supertile

# P (product/scatter) column layout
C_P12 = 0           # 384: (j16 x [i16 p1 | i8 p2]) unreduced
C_M13 = 384         # 24: m1 path3 (c3,j8) reduced
C_M14 = 408         # 24: m1 path4 (c3,j8) reduced
C_EW = 432          # 1: edge weight (norm channel)
PCOLS = 433

# irrep-norm scale folding: device computes rsqrt(sum of squares); the
# 1/sqrt(mean) = sqrt(16) (s) / sqrt(8) (v) factors are folded into weights
FS = 4.0
FV = float(np.sqrt(8.0))


def _host_prep(x, edge_src, edge_dst, edge_sh, edge_rbf, edge_len,
               w_r1, b_r1, w_r2, b_r2, w_g1, b_g1, w_g2, b_g2,
               Wm_s, Wm_v, Wu_s, Wu_v, Ws_s, Ws_v, res_scale):
    order = np.argsort(edge_dst, kind="stable")
    src_s = edge_src[order]
    dst_s = edge_dst[order]
    sh_s = edge_sh[order]
    rbf_s = edge_rbf[order]
    len_s = edge_len[order]

    deg = np.bincount(edge_dst, minlength=N).astype(np.int64)
    cum = np.concatenate([[0], np.cumsum(deg)])

    bounds = [0]
    for k in range(1, NCORE):
        bounds.append(int(np.searchsorted(cum, k * E // NCORE)))
    bounds.append(N)

    cores = []
    for k in range(NCORE):
        n0, n1 = bounds[k], bounds[k + 1]
        sups = []  # (node_base, estart, ecnt)
        nb = n0
        while nb < n1:
            nn = nb
            cnt = 0
            while nn < n1 and nn - nb < NPW and cnt + deg[nn] <= SUPE:
                cnt += int(deg[nn])
                nn += 1
            sups.append((nb, int(cum[nb]), cnt))
            nb = nn
        cores.append((n0, n1, sups))

    nsup = max(len(c[2]) for c in cores)

    # ---- host-transformed weights (shared across cores) ----
    w1p = np.zeros((9, 128), np.float32)
    w1p[:8, :64] = w_r1
    w1p[:8, 64:] = w_g1
    w1p[8, :64] = b_r1
    w1p[8, 64:] = b_g1

    # w2e [128, 578]: rows 0:64 = w_r2 (reordered cols), rows 64:128 zero
    # except gate col. cols: 0:384 interleaved (j16 x [i16 p1 | i8 p2]),
    # 384:512 p3 (j8,i16), 512:576 p4 (j8,i8), 576 gate, 577 pad
    # block scales fold the 1/sqrt(mean)-vs-rsqrt(sum) factors: paths
    # contracting normalized s get FS, normalized v get FV
    w2e = np.zeros((128, 578), np.float32)
    wsrc = w_r2.astype(np.float32)  # [64, 576]
    # p1: our col j*24+i <- ref col i*16+j (i16, j16)
    jj, ii = np.meshgrid(np.arange(16), np.arange(16), indexing="ij")
    w2e[:64, (jj * 24 + ii).ravel()] = FS * wsrc[:, (ii * 16 + jj).ravel()]
    # p2: our col j*24+16+i <- ref col 256+i*16+j (i8, j16)
    jj, ii = np.meshgrid(np.arange(16), np.arange(8), indexing="ij")
    w2e[:64, (jj * 24 + 16 + ii).ravel()] = FV * wsrc[:, (256 + ii * 16 + jj).ravel()]
    # p3: our col 384+j*16+i <- ref col 384+i*8+j (i16, j8)
    jj, ii = np.meshgrid(np.arange(8), np.arange(16), indexing="ij")
    w2e[:64, (384 + jj * 16 + ii).ravel()] = FS * wsrc[:, (384 + ii * 8 + jj).ravel()]
    # p4: our col 512+j*8+i <- ref col 512+i*8+j (i8, j8)
    jj, ii = np.meshgrid(np.arange(8), np.arange(8), indexing="ij")
    w2e[:64, (512 + jj * 8 + ii).ravel()] = FV * wsrc[:, (512 + ii * 8 + jj).ravel()]
    w2e[64:128, 576] = w_g2[:, 0]

    # b_r2 row, same column order and scales (only used when b_r2 != 0)
    br2e = np.zeros((1, 578), np.float32)
    bsrc = b_r2.astype(np.float32)
    jj, ii = np.meshgrid(np.arange(16), np.arange(16), indexing="ij")
    br2e[0, (jj * 24 + ii).ravel()] = FS * bsrc[(ii * 16 + jj).ravel()]
    jj, ii = np.meshgrid(np.arange(16), np.arange(8), indexing="ij")
    br2e[0, (jj * 24 + 16 + ii).ravel()] = FV * bsrc[(256 + ii * 16 + jj).ravel()]
    jj, ii = np.meshgrid(np.arange(8), np.arange(16), indexing="ij")
    br2e[0, (384 + jj * 16 + ii).ravel()] = FS * bsrc[(384 + ii * 8 + jj).ravel()]
    jj, ii = np.meshgrid(np.arange(8), np.arange(8), indexing="ij")
    br2e[0, (512 + jj * 8 + ii).ravel()] = FV * bsrc[(512 + ii * 8 + jj).ravel()]
    use_bias = bool(np.any(b_r2 != 0.0))

    s0 = 1.0 / math.sqrt(MUL0)
    s1 = 1.0 / math.sqrt(MUL1)
    wms = (Wm_s * s0).astype(np.float32)                      # [16,24]
    wmv = np.zeros((24, 24), np.float32)
    wuv = np.zeros((24, 24), np.float32)
    wsv = np.zeros((24, 24), np.float32)
    for c in range(3):
        for j in range(8):
            for j2 in range(8):
                wmv[c * 8 + j, c * 8 + j2] = Wm_v[j, j2] * s1
                wuv[c * 8 + j, j2 * 3 + c] = Wu_v[j, j2] * s1
                wsv[j * 3 + c, j2 * 3 + c] = Ws_v[j, j2] * s1 * FV
    wus = (Wu_s * s0).astype(np.float32)
    wss = (Ws_s * s0 * FS).astype(np.float32)
    rep = np.zeros((8, 24), np.float32)
    for c in range(3):
        for j in range(8):
            rep[j, c * 8 + j] = 1.0

    shared = dict(
        w1p=w1p.astype(BF), w2e=w2e.astype(BF), br2e=br2e.astype(BF),
        wms=wms.astype(BF), wmv=wmv.astype(BF), rep=rep.astype(BF),
        wus=wus.astype(BF), wss=wss.astype(BF),
        wuv=wuv.astype(BF), wsv=wsv.astype(BF))

    in_maps = []
    metas = []
    for k in range(NCORE):
        n0, n1, sups = cores[k]
        ns = len(sups)
        idx = np.full((nsup, SUPE), -1, np.int64)
        base_arr = np.full((nsup,), n1, np.int64)
        span_arr = np.zeros((nsup,), np.int64)
        for si, (nb, es, cnt) in enumerate(sups):
            idx[si, :cnt] = np.arange(es, es + cnt)
            base_arr[si] = nb
            span_arr[si] = min(NPW, n1 - nb)
        mask = idx >= 0
        ic = np.clip(idx, 0, E - 1)

        feat = x[src_s[ic]]                                    # [nsup,SUPE,40]
        shp = sh_s[ic].astype(np.float32)
        lenp = len_s[ic].astype(np.float32)
        cw = 0.5 * (np.cos(np.pi * lenp / CUTOFF) + 1.0) * (lenp < CUTOFF)
        cwh = np.where(mask, 0.5 * cw, 0.0).astype(np.float32)  # [nsup,SUPE]
        rbfp = np.where(mask[..., None], rbf_s[ic], 0.0).astype(np.float32)
        dstl = np.where(mask, dst_s[ic] - base_arr[:, None], 0).astype(np.int64)

        # scal cols: sh0*APATH, sh1*APATH (3), sh1*APATH*INV3 (3), cwh
        scal = np.concatenate(
            [APATH * shp[..., 0:1], APATH * shp[..., 1:4],
             (APATH * INV3) * shp[..., 1:4], cwh[..., None]],
            axis=-1).astype(np.float32)                         # [nsup,SUPE,8]

        # swizzle [nsup, SUPE, F] -> [nsup, 128, SPS, F]
        def sw(a, dt):
            f = a.shape[-1]
            return np.ascontiguousarray(
                a.reshape(nsup, SPS, SUB, f).transpose(0, 2, 1, 3)).astype(dt)

        rbft = np.concatenate(
            [rbfp.reshape(nsup * 2, 512, 8).transpose(0, 2, 1),
             np.ones((nsup * 2, 1, 512), np.float32)], axis=1)  # [2nsup,9,512]

        # one-hot scatter matrices [nsup, SPS, SUB, NPW] -> [nsup,128,SPS*128]
        sel = np.zeros((nsup, SPS, SUB, NPW), np.float32)
        si_i, e_i = np.nonzero(mask)
        t_i = e_i // SUB
        p_i = e_i % SUB
        sel[si_i, t_i, p_i, dstl[si_i, e_i]] = 1.0
        sel = np.ascontiguousarray(
            sel.transpose(0, 2, 1, 3).reshape(nsup, SUB, SPS * NPW)).astype(BF)

        nodes = np.clip(base_arr[:, None] + np.arange(NPW)[None, :], 0, N - 1)
        xown = x[nodes].astype(np.float32)                      # [nsup,128,40]

        m = dict(shared)
        m.update(feat=sw(feat, BF), scal=sw(scal, np.float32),
                 rbft=np.ascontiguousarray(rbft).astype(BF), sel=sel,
                 xown=np.ascontiguousarray(xown))
        in_maps.append(m)
        metas.append((n0, n1, base_arr, span_arr, ns))

    return in_maps, metas, nsup, float(b_g2[0]), float(res_scale), use_bias


def _newton_rsqrt(nc, y, r, rh, w, msq):
    """y = 1/sqrt(msq) (all args APs of equal shape; r/rh/w scratch).
    msq is a sum of >=1 squared N(0,1) draws (roughly [1, 64]);
    r = 1/msq in ~[0.015, 1]; y = sqrt(r) by Heron from y0 = r + 0.25."""
    nc.vector.reciprocal(out=r, in_=msq)
    nc.vector.tensor_scalar_mul(out=rh, in0=r, scalar1=0.5)
    nc.vector.tensor_scalar(out=y, in0=r, scalar1=0.25, scalar2=None,
                            op0=OP.add)
    for _ in range(2):
        nc.vector.reciprocal(out=w, in_=y)
        nc.vector.tensor_tensor(out=w, in0=w, in1=rh, op=OP.mult)
        nc.vector.scalar_tensor_tensor(out=y, in0=y, scalar=0.5,
                                       in1=w, op0=OP.mult, op1=OP.add)


def build_program(nsup, bg2, res, use_bias):
    import concourse.bacc as bacc
    nc = bacc.Bacc("TRN2", target_bir_lowering=False, debug=False,
                   num_devices=NCORE)

    feat_d = nc.dram_tensor("feat", [nsup, 128, SPS, 40], BF16, kind="ExternalInput")
    scal_d = nc.dram_tensor("scal", [nsup, 128, SPS, 8], F32, kind="ExternalInput")
    rbft_d = nc.dram_tensor("rbft", [nsup * 2, 9, 512], BF16, kind="ExternalInput")
    sel_d = nc.dram_tensor("sel", [nsup, 128, SPS * 128], BF16, kind="ExternalInput")
    xown_d = nc.dram_tensor("xown", [nsup, 128, 40], F32, kind="ExternalInput")
    w1p_d = nc.dram_tensor("w1p", [9, 128], BF16, kind="ExternalInput")
    w2e_d = nc.dram_tensor("w2e", [128, 578], BF16, kind="ExternalInput")
    br2e_d = nc.dram_tensor("br2e", [1, 578], BF16, kind="ExternalInput")
    wms_d = nc.dram_tensor("wms", [16, 24], BF16, kind="ExternalInput")
    wmv_d = nc.dram_tensor("wmv", [24, 24], BF16, kind="ExternalInput")
    rep_d = nc.dram_tensor("rep", [8, 24], BF16, kind="ExternalInput")
    wus_d = nc.dram_tensor("wus", [16, 16], BF16, kind="ExternalInput")
    wss_d = nc.dram_tensor("wss", [16, 16], BF16, kind="ExternalInput")
    wuv_d = nc.dram_tensor("wuv", [24, 24], BF16, kind="ExternalInput")
    wsv_d = nc.dram_tensor("wsv", [24, 24], BF16, kind="ExternalInput")
    out_d = nc.dram_tensor("out", [nsup, 128, 40], F32, kind="ExternalOutput")

    with tile.TileContext(nc) as tc:
        with (
            tc.tile_pool(name="const", bufs=1) as cp,
            tc.tile_pool(name="io", bufs=3) as iop,
            tc.tile_pool(name="mid", bufs=3) as mp,
            tc.tile_pool(name="pp", bufs=3) as ppp,
            tc.tile_pool(name="nd", bufs=2) as ndp,
            tc.tile_pool(name="psh", bufs=2, space="PSUM") as psH,
            tc.tile_pool(name="psw0", bufs=2, space="PSUM") as psW0,
            tc.tile_pool(name="psa", bufs=2, space="PSUM") as psA,
        ):
            w1p = cp.tile([9, 128], BF16, tag="w1p")
            w2e = cp.tile([128, 578], BF16, tag="w2e")
            br2e = cp.tile([1, 578], BF16, tag="br2e")
            wms = cp.tile([16, 24], BF16, tag="wms")
            wmv = cp.tile([24, 24], BF16, tag="wmv")
            rep = cp.tile([8, 24], BF16, tag="rep")
            wus = cp.tile([16, 16], BF16, tag="wus")
            wss = cp.tile([16, 16], BF16, tag="wss")
            wuv = cp.tile([24, 24], BF16, tag="wuv")
            wsv = cp.tile([24, 24], BF16, tag="wsv")
            ident = cp.tile([128, 128], F32, tag="ident")
            for t, d in [(w1p, w1p_d), (w2e, w2e_d), (br2e, br2e_d),
                         (wms, wms_d), (wmv, wmv_d), (rep, rep_d),
                         (wus, wus_d), (wss, wss_d), (wuv, wuv_d),
                         (wsv, wsv_d)]:
                nc.sync.dma_start(out=t[:], in_=d[:])
            make_identity(nc, ident[:])
            cbg2h = cp.tile([128, 1], F32, tag="cbg2h")
            nc.gpsimd.memset(cbg2h[:], 0.5 * bg2)
            onesr = cp.tile([1, 128], BF16, tag="onesr")
            nc.gpsimd.memset(onesr[:], 1.0)

            def tposed(src_ap, rows, tag):
                tp = psH.tile([rows, 128], F32, tag="h")
                dst = ndp.tile([rows, 128], BF16, tag=tag)
                nc.tensor.transpose(out=tp[:], in_=src_ap, identity=ident[:])
                nc.scalar.copy(out=dst[:], in_=tp[:])
                return dst

            def emit_node(s_idx, agg, xo, inv):
                m0 = ndp.tile([128, 16], F32, tag="m0")
                nc.vector.reduce_sum(
                    out=m0[:],
                    in_=agg[:, 0:384].rearrange("p (j i) -> p j i", i=24),
                    axis=mybir.AxisListType.X)
                v1 = ndp.tile([128, 24], F32, tag="v1")
                nc.vector.reduce_sum(
                    out=v1[:],
                    in_=agg[:, C_M13:C_M13 + 48].rearrange(
                        "p (a b) -> p b a", b=24),
                    axis=mybir.AxisListType.X)
                nrm = ndp.tile([128, 1], F32, tag="nrm")
                nc.vector.tensor_scalar_max(out=nrm[:], in0=agg[:, C_EW, None],
                                            scalar1=EPS)
                rinv = ndp.tile([128, 1], F32, tag="rinv")
                nc.vector.reciprocal(out=rinv[:], in_=nrm[:])

                cat_s = ndp.tile([128, 32], F32, tag="cat_s")
                cat_v = ndp.tile([128, 48], F32, tag="cat_v")
                nc.gpsimd.tensor_tensor(
                    out=cat_s[:, 0:16], in0=m0[:],
                    in1=rinv[:].to_broadcast([128, 16]), op=OP.mult)
                nc.gpsimd.tensor_tensor(
                    out=cat_v[:, 0:24], in0=v1[:],
                    in1=rinv[:].to_broadcast([128, 24]), op=OP.mult)
                nc.gpsimd.tensor_tensor(
                    out=cat_s[:, 16:32], in0=xo[:, 0:16],
                    in1=inv[:, 2, 0:1].to_broadcast([128, 16]), op=OP.mult)
                nc.gpsimd.tensor_tensor(
                    out=cat_v[:, 24:48], in0=xo[:, 16:40],
                    in1=inv[:, 2, 1:2].to_broadcast([128, 24]), op=OP.mult)

                aggT_s = tposed(cat_s[:, 0:16], 16, "aTs")
                xnT_s = tposed(cat_s[:, 16:32], 16, "xnTs")
                aggT_v = tposed(cat_v[:, 0:24], 24, "aTv")
                xnT_v = tposed(cat_v[:, 24:48], 24, "xnTv")

                scp = psH.tile([16, 128], F32, tag="h")
                nc.tensor.matmul(out=scp[:], lhsT=wms[:, 0:16], rhs=aggT_s[:],
                                 start=True, stop=True)
                scalT = ndp.tile([16, 128], BF16, tag="scalT")
                nc.scalar.activation(out=scalT[:], in_=scp[:], func=AF.Silu)
                gcp = psH.tile([8, 128], F32, tag="h")
                nc.tensor.matmul(out=gcp[:], lhsT=wms[:, 16:24], rhs=aggT_s[:],
                                 start=True, stop=True)
                gtt = ndp.tile([8, 128], BF16, tag="gtt")
                nc.scalar.activation(out=gtt[:], in_=gcp[:], func=AF.Tanh,
                                     scale=0.5)
                gT = ndp.tile([8, 128], BF16, tag="gT")
                nc.scalar.activation(out=gT[:], in_=gtt[:], func=AF.Copy,
                                     scale=0.5, bias=0.5)

                vvp = psH.tile([24, 128], F32, tag="h")
                nc.tensor.matmul(out=vvp[:], lhsT=wmv[:], rhs=aggT_v[:],
                                 start=True, stop=True)
                grp = psH.tile([24, 128], F32, tag="h")
                nc.tensor.matmul(out=grp[:], lhsT=rep[:], rhs=gT[:],
                                 start=True, stop=True)
                vvc = ndp.tile([24, 128], BF16, tag="vvc")
                nc.scalar.copy(out=vvc[:], in_=vvp[:])
                vgT = ndp.tile([24, 128], BF16, tag="vgT")
                nc.vector.tensor_tensor(out=vgT[:], in0=vvc[:], in1=grp[:],
                                        op=OP.mult)

                osp = psH.tile([16, 128], F32, tag="h")
                nc.tensor.matmul(out=osp[:], lhsT=wus[:], rhs=scalT[:],
                                 start=True, stop=False)
                nc.tensor.matmul(out=osp[:], lhsT=wss[:], rhs=xnT_s[:],
                                 start=False, stop=True)
                ovp = psH.tile([24, 128], F32, tag="h")
                nc.tensor.matmul(out=ovp[:], lhsT=wuv[:], rhs=vgT[:],
                                 start=True, stop=False)
                nc.tensor.matmul(out=ovp[:], lhsT=wsv[:], rhs=xnT_v[:],
                                 start=False, stop=True)

                fTs = ndp.tile([16, 128], F32, tag="fTs")
                nc.scalar.activation(out=fTs[:], in_=osp[:], func=AF.Copy,
                                     scale=res)
                fTv = ndp.tile([24, 128], F32, tag="fTv")
                nc.scalar.activation(out=fTv[:], in_=ovp[:], func=AF.Copy,
                                     scale=res)
                fps = psH.tile([128, 16], F32, tag="h")
                nc.tensor.transpose(out=fps[:], in_=fTs[:],
                                    identity=ident[0:16, 0:16])
                fpv = psH.tile([128, 24], F32, tag="h")
                nc.tensor.transpose(out=fpv[:], in_=fTv[:],
                                    identity=ident[0:24, 0:24])
                outt = ndp.tile([128, 40], F32, tag="outt")
                nc.vector.tensor_tensor(out=outt[:, 0:16], in0=xo[:, 0:16],
                                        in1=fps[:], op=OP.add)
                nc.vector.tensor_tensor(out=outt[:, 16:40], in0=xo[:, 16:40],
                                        in1=fpv[:], op=OP.add)
                nc.sync.dma_start(out=out_d[s_idx], in_=outt[:])

            pend = None
            for s in range(nsup):
                feats = iop.tile([128, SPS, 40], BF16, tag="feat")
                scals = iop.tile([128, SPS, 8], F32, tag="scal")
                selt = iop.tile([128, SPS, 128], BF16, tag="sel")
                xo = iop.tile([128, 40], F32, tag="xo")
                nc.sync.dma_start(out=feats[:], in_=feat_d[s])
                nc.sync.dma_start(out=scals[:], in_=scal_d[s])
                nc.sync.dma_start(out=selt[:], in_=sel_d[s])
                nc.sync.dma_start(out=xo[:], in_=xown_d[s])

                # ---- joint RMS factors (raw sums of squares; mean-scales are
                # folded into w2e/wss/wsv on host) ----
                sq = mp.tile([128, SPS, 40], F32, tag="sq")
                nc.gpsimd.tensor_tensor(out=sq[:], in0=feats[:], in1=feats[:],
                                        op=OP.mult)
                xsq = mp.tile([128, 40], F32, tag="xsq")
                nc.gpsimd.tensor_tensor(out=xsq[:], in0=xo[:], in1=xo[:],
                                        op=OP.mult)
                # rows: 0 edge-s, 1 edge-v, 2 node ([s, v] in cols 0:2)
                ms = mp.tile([128, 3, SPS], F32, tag="ms")
                nc.vector.memset(ms[:, 2, 2:SPS], 1.0)
                nc.vector.reduce_sum(out=ms[:, 0, :], in_=sq[:, :, 0:16],
                                     axis=mybir.AxisListType.X)
                nc.vector.reduce_sum(out=ms[:, 1, :], in_=sq[:, :, 16:40],
                                     axis=mybir.AxisListType.X)
                nc.vector.reduce_sum(out=ms[:, 2, 0:1], in_=xsq[:, None, 0:16],
                                     axis=mybir.AxisListType.X)
                nc.vector.reduce_sum(out=ms[:, 2, 1:2], in_=xsq[:, None, 16:40],
                                     axis=mybir.AxisListType.X)
                inv = mp.tile([128, 3, SPS], F32, tag="inv")
                nr = mp.tile([128, 3, SPS], F32, tag="nr")
                nrh = mp.tile([128, 3, SPS], F32, tag="nrh")
                nw = mp.tile([128, 3, SPS], F32, tag="nw")
                _newton_rsqrt(nc, inv[:], nr[:], nrh[:], nw[:], ms[:])
                # inv rows: 0 = edge-s, 1 = edge-v, 2 = [node-s, node-v, ...]

                st = mp.tile([128, SPS, 16], BF16, tag="st")
                vt = mp.tile([128, SPS, 24], BF16, tag="vt")
                nc.gpsimd.tensor_tensor(
                    out=st[:], in0=feats[:, :, 0:16],
                    in1=inv[:, 0, :, None].to_broadcast([128, SPS, 16]),
                    op=OP.mult)
                nc.gpsimd.tensor_tensor(
                    out=vt[:], in0=feats[:, :, 16:40],
                    in1=inv[:, 1, :, None].to_broadcast([128, SPS, 24]),
                    op=OP.mult)

                # ---- radial MLP hidden for both groups ----
                hsil = []
                for g in range(2):
                    rbft = iop.tile([9, 512], BF16, tag="rbft")
                    nc.sync.dma_start(out=rbft[:], in_=rbft_d[s * 2 + g])
                    hp = psH.tile([128, 512], F32, tag="h")
                    nc.tensor.matmul(out=hp[:], lhsT=w1p[:], rhs=rbft[:],
                                     start=True, stop=True)
                    hs = mp.tile([128, 512], BF16, tag=f"hs{g}")
                    nc.scalar.activation(out=hs[:], in_=hp[:], func=AF.Silu)
                    hsil.append(hs)

                # supertile-wide chain/product tiles
                gw8 = mp.tile([128, SPS], F32, tag="gw8")
                o4 = mp.tile([128, SPS], BF16, tag="o4")
                o3cs = mp.tile([128, SPS, 6], BF16, tag="o3cs")
                i4 = mp.tile([128, SPS], BF16, tag="i4")
                g12 = mp.tile([128, SPS, 24], BF16, tag="g12")
                g4 = mp.tile([128, SPS, 24], BF16, tag="g4")
                u3 = mp.tile([128, SPS, 8], BF16, tag="u3")
                a2 = ppp.tile([128, SPS, 8, 3], BF16, tag="a2")
                t3d = ppp.tile([128, SPS, 8, 16], BF16, tag="t3")
                t4d = ppp.tile([128, SPS, 3, 64], BF16, tag="t4")
                P = ppp.tile([128, SPS, PCOLS], BF16, tag="P")

                agg = psA.tile([128, PCOLS], F32, tag="agg")
                for g in range(2):
                    sl4 = slice(g * 4, g * 4 + 4)
                    # gate+p4 matmuls first so the scalar chain overlaps the
                    # big weight matmuls that follow
                    pw1 = psH.tile([128, 4, 66], F32, tag="h")
                    for tl in range(4):
                        lhs = hsil[g][:, tl * 128:(tl + 1) * 128]
                        if use_bias:
                            nc.tensor.matmul(out=pw1[:, tl, :], lhsT=onesr[:],
                                             rhs=br2e[:, 512:578],
                                             start=True, stop=False)
                        nc.tensor.matmul(out=pw1[:, tl, :], lhsT=lhs,
                                         rhs=w2e[:, 512:578],
                                         start=not use_bias, stop=True)

                    # ---- per-edge scalar chain (gpsimd; group batch) ----
                    nc.scalar.activation(out=gw8[:, sl4], in_=pw1[:, :, 64],
                                         func=AF.Tanh, scale=0.5, bias=cbg2h[:])
                    # ew = (tanh+1)*cwh, written straight into P's norm col
                    nc.vector.scalar_tensor_tensor(
                        out=P[:, sl4, C_EW], in0=gw8[:, sl4], scalar=1.0,
                        in1=scals[:, sl4, 7], op0=OP.add, op1=OP.mult)
                    ew = P[:, sl4, C_EW]
                    nc.gpsimd.tensor_tensor(out=o4[:, sl4], in0=ew,
                                            in1=scals[:, sl4, 0], op=OP.mult)
                    nc.gpsimd.tensor_tensor(
                        out=o3cs[:, sl4, :], in0=scals[:, sl4, 1:7],
                        in1=ew[:, :, None].to_broadcast([128, 4, 6]), op=OP.mult)
                    nc.gpsimd.tensor_tensor(out=i4[:, sl4], in0=o4[:, sl4],
                                            in1=inv[:, 0, sl4], op=OP.mult)
                    nc.gpsimd.tensor_tensor(
                        out=g12[:, sl4, 0:16], in0=feats[:, sl4, 0:16],
                        in1=i4[:, sl4, None].to_broadcast([128, 4, 16]),
                        op=OP.mult)
                    nc.gpsimd.tensor_tensor(
                        out=g4[:, sl4, :], in0=vt[:, sl4, :],
                        in1=o4[:, sl4, None].to_broadcast([128, 4, 24]),
                        op=OP.mult)
                    nc.gpsimd.tensor_tensor(
                        out=a2[:, sl4],
                        in0=vt[:, sl4, :].rearrange("p s (i c) -> p s i c", c=3),
                        in1=o3cs[:, sl4, None, 3:6].to_broadcast([128, 4, 8, 3]),
                        op=OP.mult)
                    with nc.allow_low_precision(reason="3-term bf16 sum"):
                        nc.vector.reduce_sum(out=g12[:, sl4, 16:24],
                                             in_=a2[:, sl4],
                                             axis=mybir.AxisListType.X)

                    # ---- weight matmuls in double-buffered pairs, with the
                    # products for each pair issued as soon as it lands ----
                    for k in range(2):
                        sl2 = slice(g * 4 + k * 2, g * 4 + k * 2 + 2)
                        pw0 = psW0.tile([128, 2, 512], F32, tag="pw0")
                        for tl2 in range(2):
                            tl = k * 2 + tl2
                            lhs = hsil[g][:, tl * 128:(tl + 1) * 128]
                            if use_bias:
                                nc.tensor.matmul(out=pw0[:, tl2, :],
                                                 lhsT=onesr[:],
                                                 rhs=br2e[:, 0:512],
                                                 start=True, stop=False)
                            nc.tensor.matmul(out=pw0[:, tl2, :], lhsT=lhs,
                                             rhs=w2e[:, 0:512],
                                             start=not use_bias, stop=True)
                        # stage the pair to SBUF bf16 (ACT) so the products
                        # run in the fast all-SBUF 16-bit DVE mode
                        pwS = mp.tile([128, 2, 512], BF16, tag="pwS")
                        nc.scalar.copy(out=pwS[:], in_=pw0[:])
                        nc.vector.tensor_tensor(
                            out=P[:, sl2, 0:384].rearrange(
                                "p s (j i) -> p s j i", i=24),
                            in0=pwS[:, :, 0:384].rearrange(
                                "p s (j i) -> p s j i", i=24),
                            in1=g12[:, sl2, None, :].to_broadcast(
                                [128, 2, 16, 24]),
                            op=OP.mult)
                        nc.vector.tensor_tensor(
                            out=t3d[:, sl2],
                            in0=pwS[:, :, 384:512].rearrange(
                                "p s (j i) -> p s j i", i=16),
                            in1=st[:, sl2, None, :].to_broadcast(
                                [128, 2, 8, 16]),
                            op=OP.mult)
                    with nc.allow_low_precision(reason="16-term bf16 sum"):
                        nc.vector.reduce_sum(out=u3[:, sl4], in_=t3d[:, sl4],
                                             axis=mybir.AxisListType.X)
                    # stage p4 block to SBUF (ACT) so t4 runs in fast DVE mode
                    p4s = mp.tile([128, 4, 64], BF16, tag="p4s")
                    nc.scalar.copy(out=p4s[:], in_=pw1[:, :, 0:64])
                    g4r = g4[:, sl4, :].rearrange("p s (i c) -> p s i c", c=3)
                    for c in range(3):
                        nc.gpsimd.tensor_tensor(
                            out=t4d[:, sl4, c, :].rearrange(
                                "p s (j i) -> p s j i", i=8),
                            in0=p4s[:].rearrange("p s (j i) -> p s j i", i=8),
                            in1=g4r[:, :, :, c][:, :, None, :].to_broadcast(
                                [128, 4, 8, 8]),
                            op=OP.mult)
                    nc.gpsimd.tensor_tensor(
                        out=P[:, sl4, C_M13:C_M13 + 24].rearrange(
                            "p s (c j) -> p s c j", j=8),
                        in0=u3[:, sl4, None, :].to_broadcast([128, 4, 3, 8]),
                        in1=o3cs[:, sl4, 0:3, None].to_broadcast([128, 4, 3, 8]),
                        op=OP.mult)
                    with nc.allow_low_precision(reason="8-term bf16 sum"):
                        nc.vector.reduce_sum(
                            out=P[:, sl4, C_M14:C_M14 + 24],
                            in_=t4d[:, sl4].rearrange(
                                "p s c (j i) -> p s (c j) i", i=8),
                            axis=mybir.AxisListType.X)
                    # scatter this group's subtiles while the next group's
                    # matmuls proceed
                    for tl in range(4):
                        t = g * 4 + tl
                        nc.tensor.matmul(out=agg[:], lhsT=selt[:, t, :],
                                         rhs=P[:, t, :],
                                         start=(t == 0), stop=(t == SPS - 1))

                # node phase is software-pipelined by one supertile: emit the
                # PREVIOUS supertile's node phase here so it fills engine gaps
                # while this supertile's edge phase runs
                if pend is not None:
                    emit_node(*pend)
                pend = (s, agg, xo, inv)
            emit_node(*pend)

    nc.compile()
    return nc


_CACHE = {}


def kernel(**inputs):
    in_maps, metas, nsup, bg2, res, use_bias = _host_prep(**inputs)
    key = (nsup, bg2, res, use_bias)
    if key not in _CACHE:
        _CACHE[key] = build_program(nsup, bg2, res, use_bias)
    nc = _CACHE[key]
    r = run_bass_kernel_spmd(nc, in_maps, list(range(NCORE)))
    out = np.zeros((N, 40), np.float32)
    for k in range(NCORE):
        n0, n1, base_arr, span_arr, ns = metas[k]
        ob = r.results[k]["out"]
        for si in range(ns):
            sp = int(span_arr[si])
            if sp > 0:
                b = int(base_arr[si])
                out[b:b + sp] = ob[si, :sp]
    return out


# revision 46
# speedup vs baseline: 1.2303x; 1.0312x over previous
"""Trainium2 Bass kernel for nn_EquivariantInteractionBlock.

Strategy (edge/graph parallel, 8 cores):
- Host: sort edges by dst; split into 8 node-aligned contiguous ranges with
  ~E/8 edges each. Per core, pack edges into supertiles: <=1024 edges
  covering a window of <=128 consecutive dst nodes. Host gathers raw x rows
  by edge_src, precomputes the cosine cutoff, builds one-hot scatter
  matrices, and pre-swizzles everything into DMA-friendly bf16 layouts.
- Device per supertile (all matmuls bf16, fp32 PSUM accumulate):
  * radial MLP hidden: h = silu(rbf@W1) via one matmul + one silu per
    512-edge group (msg+gate hidden together, feature-major)
  * per-edge TP weights + gate logit: per 128-edge subtile one stationary
    load (h slice) and two matmuls streaming 512+66 weight columns
  * sigmoid via tanh (same ACT table set as silu -> no table reloads),
    rsqrt for RMS norms via DVE Newton iteration
  * tensor-product products on VectorE, i-reductions for paths 1/2 ride
    the scatter matmul as extra columns
  * scatter-add via host-built one-hot selection matrices (bf16 matmul)
  * node phase: normalize, two packed PE transposes, small accumulating
    matmuls for msg/update/self linears, residual in fp32
- Each core owns a disjoint node range: no collectives; host concatenates
  per-core output rows.
"""

import math
import numpy as np
import ml_dtypes

import concourse.bass as bass
import concourse.mybir as mybir
import concourse.tile as tile
from concourse.bass_utils import run_bass_kernel_spmd
from concourse.masks import make_identity

F32 = mybir.dt.float32
BF16 = mybir.dt.bfloat16
AF = mybir.ActivationFunctionType
OP = mybir.AluOpType
BF = ml_dtypes.bfloat16

N = 50000
E = 400000
MUL0 = 16
MUL1 = 8
RBF = 8
HID = 64
CUTOFF = 5.0
EPS = 1e-8
INV3 = float(1.0 / np.sqrt(np.float32(3.0)))
APATH = float(1.0 / math.sqrt(MUL0 + MUL1))
NCORE = 8
SUB = 128          # edges per subtile
SPS = 8            # subtiles per supertile
SUPE = SUB * SPS   # 1024 edges per supertile
NPW = 128          # node window per supertile

# P (product/scatter) column layout
C_P12 = 0           # 384: (j16 x [i16 p1 | i8 p2]) unreduced
C_M13 = 384         # 24: m1 path3 (c3,j8) reduced
C_M14 = 408         # 24: m1 path4 (c3,j8) reduced
C_EW = 432          # 1: edge weight (norm channel)
PCOLS = 433

# irrep-norm scale folding: device computes rsqrt(sum of squares); the
# 1/sqrt(mean) = sqrt(16) (s) / sqrt(8) (v) factors are folded into weights
FS = 4.0
FV = float(np.sqrt(8.0))


def _host_prep(x, edge_src, edge_dst, edge_sh, edge_rbf, edge_len,
               w_r1, b_r1, w_r2, b_r2, w_g1, b_g1, w_g2, b_g2,
               Wm_s, Wm_v, Wu_s, Wu_v, Ws_s, Ws_v, res_scale):
    order = np.argsort(edge_dst, kind="stable")
    src_s = edge_src[order]
    dst_s = edge_dst[order]
    sh_s = edge_sh[order]
    rbf_s = edge_rbf[order]
    len_s = edge_len[order]

    deg = np.bincount(edge_dst, minlength=N).astype(np.int64)
    cum = np.concatenate([[0], np.cumsum(deg)])

    bounds = [0]
    for k in range(1, NCORE):
        bounds.append(int(np.searchsorted(cum, k * E // NCORE)))
    bounds.append(N)

    cores = []
    for k in range(NCORE):
        n0, n1 = bounds[k], bounds[k + 1]
        sups = []  # (node_base, estart, ecnt)
        nb = n0
        while nb < n1:
            nn = nb
            cnt = 0
            while nn < n1 and nn - nb < NPW and cnt + deg[nn] <= SUPE:
                cnt += int(deg[nn])
                nn += 1
            sups.append((nb, int(cum[nb]), cnt))
            nb = nn
        cores.append((n0, n1, sups))

    nsup = max(len(c[2]) for c in cores)

    # ---- host-transformed weights (shared across cores) ----
    w1p = np.zeros((9, 128), np.float32)
    w1p[:8, :64] = w_r1
    w1p[:8, 64:] = w_g1
    w1p[8, :64] = b_r1
    w1p[8, 64:] = b_g1

    # w2e [128, 578]: rows 0:64 = w_r2 (reordered cols), rows 64:128 zero
    # except gate col. cols: 0:384 interleaved (j16 x [i16 p1 | i8 p2]),
    # 384:512 p3 (j8,i16), 512:576 p4 (j8,i8), 576 gate, 577 pad
    # block scales fold the 1/sqrt(mean)-vs-rsqrt(sum) factors: paths
    # contracting normalized s get FS, normalized v get FV
    w2e = np.zeros((128, 578), np.float32)
    wsrc = w_r2.astype(np.float32)  # [64, 576]
    # p1: our col j*24+i <- ref col i*16+j (i16, j16)
    jj, ii = np.meshgrid(np.arange(16), np.arange(16), indexing="ij")
    w2e[:64, (jj * 24 + ii).ravel()] = FS * wsrc[:, (ii * 16 + jj).ravel()]
    # p2: our col j*24+16+i <- ref col 256+i*16+j (i8, j16)
    jj, ii = np.meshgrid(np.arange(16), np.arange(8), indexing="ij")
    w2e[:64, (jj * 24 + 16 + ii).ravel()] = FV * wsrc[:, (256 + ii * 16 + jj).ravel()]
    # p3: our col 384+j*16+i <- ref col 384+i*8+j (i16, j8)
    jj, ii = np.meshgrid(np.arange(8), np.arange(16), indexing="ij")
    w2e[:64, (384 + jj * 16 + ii).ravel()] = FS * wsrc[:, (384 + ii * 8 + jj).ravel()]
    # p4: our col 512+j*8+i <- ref col 512+i*8+j (i8, j8)
    jj, ii = np.meshgrid(np.arange(8), np.arange(8), indexing="ij")
    w2e[:64, (512 + jj * 8 + ii).ravel()] = FV * wsrc[:, (512 + ii * 8 + jj).ravel()]
    w2e[64:128, 576] = w_g2[:, 0]

    # b_r2 row, same column order and scales (only used when b_r2 != 0)
    br2e = np.zeros((1, 578), np.float32)
    bsrc = b_r2.astype(np.float32)
    jj, ii = np.meshgrid(np.arange(16), np.arange(16), indexing="ij")
    br2e[0, (jj * 24 + ii).ravel()] = FS * bsrc[(ii * 16 + jj).ravel()]
    jj, ii = np.meshgrid(np.arange(16), np.arange(8), indexing="ij")
    br2e[0, (jj * 24 + 16 + ii).ravel()] = FV * bsrc[(256 + ii * 16 + jj).ravel()]
    jj, ii = np.meshgrid(np.arange(8), np.arange(16), indexing="ij")
    br2e[0, (384 + jj * 16 + ii).ravel()] = FS * bsrc[(384 + ii * 8 + jj).ravel()]
    jj, ii = np.meshgrid(np.arange(8), np.arange(8), indexing="ij")
    br2e[0, (512 + jj * 8 + ii).ravel()] = FV * bsrc[(512 + ii * 8 + jj).ravel()]
    use_bias = bool(np.any(b_r2 != 0.0))

    s0 = 1.0 / math.sqrt(MUL0)
    s1 = 1.0 / math.sqrt(MUL1)
    wms = (Wm_s * s0).astype(np.float32)                      # [16,24]
    wmv = np.zeros((24, 24), np.float32)
    wuv = np.zeros((24, 24), np.float32)
    wsv = np.zeros((24, 24), np.float32)
    for c in range(3):
        for j in range(8):
            for j2 in range(8):
                wmv[c * 8 + j, c * 8 + j2] = Wm_v[j, j2] * s1
                wuv[c * 8 + j, j2 * 3 + c] = Wu_v[j, j2] * s1
                wsv[j * 3 + c, j2 * 3 + c] = Ws_v[j, j2] * s1 * FV
    wus = (Wu_s * s0).astype(np.float32)
    wss = (Ws_s * s0 * FS).astype(np.float32)
    rep = np.zeros((8, 24), np.float32)
    for c in range(3):
        for j in range(8):
            rep[j, c * 8 + j] = 1.0

    shared = dict(
        w1p=w1p.astype(BF), w2e=w2e.astype(BF), br2e=br2e.astype(BF),
        wms=wms.astype(BF), wmv=wmv.astype(BF), rep=rep.astype(BF),
        wus=wus.astype(BF), wss=wss.astype(BF),
        wuv=wuv.astype(BF), wsv=wsv.astype(BF))

    in_maps = []
    metas = []
    for k in range(NCORE):
        n0, n1, sups = cores[k]
        ns = len(sups)
        idx = np.full((nsup, SUPE), -1, np.int64)
        base_arr = np.full((nsup,), n1, np.int64)
        span_arr = np.zeros((nsup,), np.int64)
        for si, (nb, es, cnt) in enumerate(sups):
            idx[si, :cnt] = np.arange(es, es + cnt)
            base_arr[si] = nb
            span_arr[si] = min(NPW, n1 - nb)
        mask = idx >= 0
        ic = np.clip(idx, 0, E - 1)

        feat = x[src_s[ic]]                                    # [nsup,SUPE,40]
        shp = sh_s[ic].astype(np.float32)
        lenp = len_s[ic].astype(np.float32)
        cw = 0.5 * (np.cos(np.pi * lenp / CUTOFF) + 1.0) * (lenp < CUTOFF)
        cwh = np.where(mask, 0.5 * cw, 0.0).astype(np.float32)  # [nsup,SUPE]
        rbfp = np.where(mask[..., None], rbf_s[ic], 0.0).astype(np.float32)
        dstl = np.where(mask, dst_s[ic] - base_arr[:, None], 0).astype(np.int64)

        # scal cols: sh0*APATH, sh1*APATH (3), sh1*APATH*INV3 (3), cwh
        scal = np.concatenate(
            [APATH * shp[..., 0:1], APATH * shp[..., 1:4],
             (APATH * INV3) * shp[..., 1:4], cwh[..., None]],
            axis=-1).astype(np.float32)                         # [nsup,SUPE,8]

        # swizzle [nsup, SUPE, F] -> [nsup, 128, SPS, F]
        def sw(a, dt):
            f = a.shape[-1]
            return np.ascontiguousarray(
                a.reshape(nsup, SPS, SUB, f).transpose(0, 2, 1, 3)).astype(dt)

        rbft = np.concatenate(
            [rbfp.reshape(nsup * 2, 512, 8).transpose(0, 2, 1),
             np.ones((nsup * 2, 1, 512), np.float32)], axis=1)  # [2nsup,9,512]

        # one-hot scatter matrices [nsup, SPS, SUB, NPW] -> [nsup,128,SPS*128]
        sel = np.zeros((nsup, SPS, SUB, NPW), np.float32)
        si_i, e_i = np.nonzero(mask)
        t_i = e_i // SUB
        p_i = e_i % SUB
        sel[si_i, t_i, p_i, dstl[si_i, e_i]] = 1.0
        sel = np.ascontiguousarray(
            sel.transpose(0, 2, 1, 3).reshape(nsup, SUB, SPS * NPW)).astype(BF)

        nodes = np.clip(base_arr[:, None] + np.arange(NPW)[None, :], 0, N - 1)
        xown = x[nodes].astype(np.float32)                      # [nsup,128,40]

        m = dict(shared)
        m.update(feat=sw(feat, BF), scal=sw(scal, np.float32),
                 rbft=np.ascontiguousarray(rbft).astype(BF), sel=sel,
                 xown=np.ascontiguousarray(xown))
        in_maps.append(m)
        metas.append((n0, n1, base_arr, span_arr, ns))

    return in_maps, metas, nsup, float(b_g2[0]), float(res_scale), use_bias


def _newton_rsqrt(nc, y, r, rh, w, msq):
    """y = 1/sqrt(msq) (all args APs of equal shape; r/rh/w scratch).
    msq is a sum of >=1 squared N(0,1) draws (roughly [1, 64]);
    r = 1/msq in ~[0.015, 1]; y = sqrt(r) by Heron from y0 = r + 0.25."""
    nc.vector.reciprocal(out=r, in_=msq)
    nc.vector.tensor_scalar_mul(out=rh, in0=r, scalar1=0.5)
    nc.vector.tensor_scalar(out=y, in0=r, scalar1=0.25, scalar2=None,
                            op0=OP.add)
    for _ in range(2):
        nc.vector.reciprocal(out=w, in_=y)
        nc.vector.tensor_tensor(out=w, in0=w, in1=rh, op=OP.mult)
        nc.vector.scalar_tensor_tensor(out=y, in0=y, scalar=0.5,
                                       in1=w, op0=OP.mult, op1=OP.add)


def build_program(nsup, bg2, res, use_bias):
    import concourse.bacc as bacc
    nc = bacc.Bacc("TRN2", target_bir_lowering=False, debug=False,
                   num_devices=NCORE)

    feat_d = nc.dram_tensor("feat", [nsup, 128, SPS, 40], BF16, kind="ExternalInput")
    scal_d = nc.dram_tensor("scal", [nsup, 128, SPS, 8], F32, kind="ExternalInput")
    rbft_d = nc.dram_tensor("rbft", [nsup * 2, 9, 512], BF16, kind="ExternalInput")
    sel_d = nc.dram_tensor("sel", [nsup, 128, SPS * 128], BF16, kind="ExternalInput")
    xown_d = nc.dram_tensor("xown", [nsup, 128, 40], F32, kind="ExternalInput")
    w1p_d = nc.dram_tensor("w1p", [9, 128], BF16, kind="ExternalInput")
    w2e_d = nc.dram_tensor("w2e", [128, 578], BF16, kind="ExternalInput")
    br2e_d = nc.dram_tensor("br2e", [1, 578], BF16, kind="ExternalInput")
    wms_d = nc.dram_tensor("wms", [16, 24], BF16, kind="ExternalInput")
    wmv_d = nc.dram_tensor("wmv", [24, 24], BF16, kind="ExternalInput")
    rep_d = nc.dram_tensor("rep", [8, 24], BF16, kind="ExternalInput")
    wus_d = nc.dram_tensor("wus", [16, 16], BF16, kind="ExternalInput")
    wss_d = nc.dram_tensor("wss", [16, 16], BF16, kind="ExternalInput")
    wuv_d = nc.dram_tensor("wuv", [24, 24], BF16, kind="ExternalInput")
    wsv_d = nc.dram_tensor("wsv", [24, 24], BF16, kind="ExternalInput")
    out_d = nc.dram_tensor("out", [nsup, 128, 40], F32, kind="ExternalOutput")

    with tile.TileContext(nc) as tc:
        with (
            tc.tile_pool(name="const", bufs=1) as cp,
            tc.tile_pool(name="io", bufs=3) as iop,
            tc.tile_pool(name="mid", bufs=3) as mp,
            tc.tile_pool(name="pp", bufs=3) as ppp,
            tc.tile_pool(name="nd", bufs=2) as ndp,
            tc.tile_pool(name="psh", bufs=2, space="PSUM") as psH,
            tc.tile_pool(name="psw0", bufs=2, space="PSUM") as psW0,
            tc.tile_pool(name="psa", bufs=2, space="PSUM") as psA,
        ):
            w1p = cp.tile([9, 128], BF16, tag="w1p")
            w2e = cp.tile([128, 578], BF16, tag="w2e")
            br2e = cp.tile([1, 578], BF16, tag="br2e")
            wms = cp.tile([16, 24], BF16, tag="wms")
            wmv = cp.tile([24, 24], BF16, tag="wmv")
            rep = cp.tile([8, 24], BF16, tag="rep")
            wus = cp.tile([16, 16], BF16, tag="wus")
            wss = cp.tile([16, 16], BF16, tag="wss")
            wuv = cp.tile([24, 24], BF16, tag="wuv")
            wsv = cp.tile([24, 24], BF16, tag="wsv")
            ident = cp.tile([128, 128], F32, tag="ident")
            for t, d in [(w1p, w1p_d), (w2e, w2e_d), (br2e, br2e_d),
                         (wms, wms_d), (wmv, wmv_d), (rep, rep_d),
                         (wus, wus_d), (wss, wss_d), (wuv, wuv_d),
                         (wsv, wsv_d)]:
                nc.sync.dma_start(out=t[:], in_=d[:])
            make_identity(nc, ident[:])
            cbg2h = cp.tile([128, 1], F32, tag="cbg2h")
            nc.gpsimd.memset(cbg2h[:], 0.5 * bg2)
            onesr = cp.tile([1, 128], BF16, tag="onesr")
            nc.gpsimd.memset(onesr[:], 1.0)

            def tposed(src_ap, rows, tag):
                tp = psH.tile([rows, 128], F32, tag="h")
                dst = ndp.tile([rows, 128], BF16, tag=tag)
                nc.tensor.transpose(out=tp[:], in_=src_ap, identity=ident[:])
                nc.scalar.copy(out=dst[:], in_=tp[:])
                return dst

            def emit_node(s_idx, agg, xo, inv):
                m0 = ndp.tile([128, 16], F32, tag="m0")
                nc.vector.reduce_sum(
                    out=m0[:],
                    in_=agg[:, 0:384].rearrange("p (j i) -> p j i", i=24),
                    axis=mybir.AxisListType.X)
                v1 = ndp.tile([128, 24], F32, tag="v1")
                nc.vector.reduce_sum(
                    out=v1[:],
                    in_=agg[:, C_M13:C_M13 + 48].rearrange(
                        "p (a b) -> p b a", b=24),
                    axis=mybir.AxisListType.X)
                nrm = ndp.tile([128, 1], F32, tag="nrm")
                nc.vector.tensor_scalar_max(out=nrm[:], in0=agg[:, C_EW, None],
                                            scalar1=EPS)
                rinv = ndp.tile([128, 1], F32, tag="rinv")
                nc.vector.reciprocal(out=rinv[:], in_=nrm[:])

                cat_s = ndp.tile([128, 32], F32, tag="cat_s")
                cat_v = ndp.tile([128, 48], F32, tag="cat_v")
                nc.gpsimd.tensor_tensor(
                    out=cat_s[:, 0:16], in0=m0[:],
                    in1=rinv[:].to_broadcast([128, 16]), op=OP.mult)
                nc.gpsimd.tensor_tensor(
                    out=cat_v[:, 0:24], in0=v1[:],
                    in1=rinv[:].to_broadcast([128, 24]), op=OP.mult)
                nc.gpsimd.tensor_tensor(
                    out=cat_s[:, 16:32], in0=xo[:, 0:16],
                    in1=inv[:, 0, SPS:SPS + 1].to_broadcast([128, 16]),
                    op=OP.mult)
                nc.gpsimd.tensor_tensor(
                    out=cat_v[:, 24:48], in0=xo[:, 16:40],
                    in1=inv[:, 1, SPS:SPS + 1].to_broadcast([128, 24]),
                    op=OP.mult)

                aggT_s = tposed(cat_s[:, 0:16], 16, "aTs")
                xnT_s = tposed(cat_s[:, 16:32], 16, "xnTs")
                aggT_v = tposed(cat_v[:, 0:24], 24, "aTv")
                xnT_v = tposed(cat_v[:, 24:48], 24, "xnTv")

                scp = psH.tile([16, 128], F32, tag="h")
                nc.tensor.matmul(out=scp[:], lhsT=wms[:, 0:16], rhs=aggT_s[:],
                                 start=True, stop=True)
                scalT = ndp.tile([16, 128], BF16, tag="scalT")
                nc.scalar.activation(out=scalT[:], in_=scp[:], func=AF.Silu)
                gcp = psH.tile([8, 128], F32, tag="h")
                nc.tensor.matmul(out=gcp[:], lhsT=wms[:, 16:24], rhs=aggT_s[:],
                                 start=True, stop=True)
                gtt = ndp.tile([8, 128], BF16, tag="gtt")
                nc.scalar.activation(out=gtt[:], in_=gcp[:], func=AF.Tanh,
                                     scale=0.5)
                gT = ndp.tile([8, 128], BF16, tag="gT")
                nc.scalar.activation(out=gT[:], in_=gtt[:], func=AF.Copy,
                                     scale=0.5, bias=0.5)

                vvp = psH.tile([24, 128], F32, tag="h")
                nc.tensor.matmul(out=vvp[:], lhsT=wmv[:], rhs=aggT_v[:],
                                 start=True, stop=True)
                grp = psH.tile([24, 128], F32, tag="h")
                nc.tensor.matmul(out=grp[:], lhsT=rep[:], rhs=gT[:],
                                 start=True, stop=True)
                vvc = ndp.tile([24, 128], BF16, tag="vvc")
                nc.scalar.copy(out=vvc[:], in_=vvp[:])
                vgT = ndp.tile([24, 128], BF16, tag="vgT")
                nc.vector.tensor_tensor(out=vgT[:], in0=vvc[:], in1=grp[:],
                                        op=OP.mult)

                osp = psH.tile([16, 128], F32, tag="h")
                nc.tensor.matmul(out=osp[:], lhsT=wus[:], rhs=scalT[:],
                                 start=True, stop=False)
                nc.tensor.matmul(out=osp[:], lhsT=wss[:], rhs=xnT_s[:],
                                 start=False, stop=True)
                ovp = psH.tile([24, 128], F32, tag="h")
                nc.tensor.matmul(out=ovp[:], lhsT=wuv[:], rhs=vgT[:],
                                 start=True, stop=False)
                nc.tensor.matmul(out=ovp[:], lhsT=wsv[:], rhs=xnT_v[:],
                                 start=False, stop=True)

                fTs = ndp.tile([16, 128], F32, tag="fTs")
                nc.scalar.activation(out=fTs[:], in_=osp[:], func=AF.Copy,
                                     scale=res)
                fTv = ndp.tile([24, 128], F32, tag="fTv")
                nc.scalar.activation(out=fTv[:], in_=ovp[:], func=AF.Copy,
                                     scale=res)
                fps = psH.tile([128, 16], F32, tag="h")
                nc.tensor.transpose(out=fps[:], in_=fTs[:],
                                    identity=ident[0:16, 0:16])
                fpv = psH.tile([128, 24], F32, tag="h")
                nc.tensor.transpose(out=fpv[:], in_=fTv[:],
                                    identity=ident[0:24, 0:24])
                outt = ndp.tile([128, 40], F32, tag="outt")
                nc.vector.tensor_tensor(out=outt[:, 0:16], in0=xo[:, 0:16],
                                        in1=fps[:], op=OP.add)
                nc.vector.tensor_tensor(out=outt[:, 16:40], in0=xo[:, 16:40],
                                        in1=fpv[:], op=OP.add)
                nc.sync.dma_start(out=out_d[s_idx], in_=outt[:])

            pend = None
            for s in range(nsup):
                feats = iop.tile([128, SPS, 40], BF16, tag="feat")
                scals = iop.tile([128, SPS, 8], F32, tag="scal")
                selt = iop.tile([128, SPS, 128], BF16, tag="sel")
                xo = iop.tile([128, 40], F32, tag="xo")
                nc.sync.dma_start(out=feats[:], in_=feat_d[s])
                nc.sync.dma_start(out=scals[:], in_=scal_d[s])
                nc.sync.dma_start(out=selt[:], in_=sel_d[s])
                nc.sync.dma_start(out=xo[:], in_=xown_d[s])

                # ---- joint RMS factors (raw sums of squares; mean-scales are
                # folded into w2e/wss/wsv on host). Col 8 of the sq/ms/inv
                # tiles carries the node (xo) statistics so the reduces and
                # Newton iteration cover edges + node in one batch. ----
                sq = mp.tile([128, SPS + 1, 40], F32, tag="sq")
                nc.gpsimd.tensor_tensor(out=sq[:, 0:SPS, :], in0=feats[:],
                                        in1=feats[:], op=OP.mult)
                nc.gpsimd.tensor_tensor(out=sq[:, SPS, :], in0=xo[:],
                                        in1=xo[:], op=OP.mult)
                # rows: 0 = s-sums, 1 = v-sums (cols 0:8 edges, col 8 node)
                ms = mp.tile([128, 2, SPS + 1], F32, tag="ms")
                nc.vector.reduce_sum(out=ms[:, 0, :], in_=sq[:, :, 0:16],
                                     axis=mybir.AxisListType.X)
                nc.vector.reduce_sum(out=ms[:, 1, :], in_=sq[:, :, 16:40],
                                     axis=mybir.AxisListType.X)
                inv = mp.tile([128, 2, SPS + 1], F32, tag="inv")
                nr = mp.tile([128, 2, SPS + 1], F32, tag="nr")
                nrh = mp.tile([128, 2, SPS + 1], F32, tag="nrh")
                nw = mp.tile([128, 2, SPS + 1], F32, tag="nw")
                _newton_rsqrt(nc, inv[:], nr[:], nrh[:], nw[:], ms[:])

                st = mp.tile([128, SPS, 16], BF16, tag="st")
                vt = mp.tile([128, SPS, 24], BF16, tag="vt")
                nc.gpsimd.tensor_tensor(
                    out=st[:], in0=feats[:, :, 0:16],
                    in1=inv[:, 0, 0:SPS, None].to_broadcast([128, SPS, 16]),
                    op=OP.mult)
                nc.gpsimd.tensor_tensor(
                    out=vt[:], in0=feats[:, :, 16:40],
                    in1=inv[:, 1, 0:SPS, None].to_broadcast([128, SPS, 24]),
                    op=OP.mult)

                # ---- radial MLP hidden for both groups ----
                hsil = []
                for g in range(2):
                    rbft = iop.tile([9, 512], BF16, tag="rbft")
                    nc.sync.dma_start(out=rbft[:], in_=rbft_d[s * 2 + g])
                    hp = psH.tile([128, 512], F32, tag="h")
                    nc.tensor.matmul(out=hp[:], lhsT=w1p[:], rhs=rbft[:],
                                     start=True, stop=True)
                    hs = mp.tile([128, 512], BF16, tag=f"hs{g}")
                    nc.scalar.activation(out=hs[:], in_=hp[:], func=AF.Silu)
                    hsil.append(hs)

                # supertile-wide chain/product tiles
                gw8 = mp.tile([128, SPS], F32, tag="gw8")
                o4 = mp.tile([128, SPS], BF16, tag="o4")
                o3cs = mp.tile([128, SPS, 6], BF16, tag="o3cs")
                i4 = mp.tile([128, SPS], BF16, tag="i4")
                g12 = mp.tile([128, SPS, 24], BF16, tag="g12")
                g4 = mp.tile([128, SPS, 24], BF16, tag="g4")
                u3 = mp.tile([128, SPS, 8], BF16, tag="u3")
                a2 = ppp.tile([128, SPS, 8, 3], BF16, tag="a2")
                t3d = ppp.tile([128, SPS, 8, 16], BF16, tag="t3")
                t4d = ppp.tile([128, SPS, 3, 64], BF16, tag="t4")
                P = ppp.tile([128, SPS, PCOLS], BF16, tag="P")

                agg = psA.tile([128, PCOLS], F32, tag="agg")
                for g in range(2):
                    sl4 = slice(g * 4, g * 4 + 4)
                    # gate+p4 matmuls first so the scalar chain overlaps the
                    # big weight matmuls that follow
                    pw1 = psH.tile([128, 4, 66], F32, tag="h")
                    for tl in range(4):
                        lhs = hsil[g][:, tl * 128:(tl + 1) * 128]
                        if use_bias:
                            nc.tensor.matmul(out=pw1[:, tl, :], lhsT=onesr[:],
                                             rhs=br2e[:, 512:578],
                                             start=True, stop=False)
                        nc.tensor.matmul(out=pw1[:, tl, :], lhsT=lhs,
                                         rhs=w2e[:, 512:578],
                                         start=not use_bias, stop=True)

                    # ---- per-edge scalar chain (gpsimd; group batch) ----
                    nc.scalar.activation(out=gw8[:, sl4], in_=pw1[:, :, 64],
                                         func=AF.Tanh, scale=0.5, bias=cbg2h[:])
                    # ew = (tanh+1)*cwh, written straight into P's norm col
                    nc.vector.scalar_tensor_tensor(
                        out=P[:, sl4, C_EW], in0=gw8[:, sl4], scalar=1.0,
                        in1=scals[:, sl4, 7], op0=OP.add, op1=OP.mult)
                    ew = P[:, sl4, C_EW]
                    nc.gpsimd.tensor_tensor(out=o4[:, sl4], in0=ew,
                                            in1=scals[:, sl4, 0], op=OP.mult)
                    nc.gpsimd.tensor_tensor(
                        out=o3cs[:, sl4, :], in0=scals[:, sl4, 1:7],
                        in1=ew[:, :, None].to_broadcast([128, 4, 6]), op=OP.mult)
                    nc.gpsimd.tensor_tensor(out=i4[:, sl4], in0=o4[:, sl4],
                                            in1=inv[:, 0, sl4], op=OP.mult)
                    nc.gpsimd.tensor_tensor(
                        out=g12[:, sl4, 0:16], in0=feats[:, sl4, 0:16],
                        in1=i4[:, sl4, None].to_broadcast([128, 4, 16]),
                        op=OP.mult)
                    nc.gpsimd.tensor_tensor(
                        out=g4[:, sl4, :], in0=vt[:, sl4, :],
                        in1=o4[:, sl4, None].to_broadcast([128, 4, 24]),
                        op=OP.mult)
                    nc.gpsimd.tensor_tensor(
                        out=a2[:, sl4],
                        in0=vt[:, sl4, :].rearrange("p s (i c) -> p s i c", c=3),
                        in1=o3cs[:, sl4, None, 3:6].to_broadcast([128, 4, 8, 3]),
                        op=OP.mult)
                    with nc.allow_low_precision(reason="3-term bf16 sum"):
                        nc.vector.reduce_sum(out=g12[:, sl4, 16:24],
                                             in_=a2[:, sl4],
                                             axis=mybir.AxisListType.X)

                    # ---- weight matmuls in double-buffered pairs, with the
                    # products for each pair issued as soon as it lands ----
                    for k in range(2):
                        sl2 = slice(g * 4 + k * 2, g * 4 + k * 2 + 2)
                        pw0 = psW0.tile([128, 2, 512], F32, tag="pw0")
                        for tl2 in range(2):
                            tl = k * 2 + tl2
                            lhs = hsil[g][:, tl * 128:(tl + 1) * 128]
                            if use_bias:
                                nc.tensor.matmul(out=pw0[:, tl2, :],
                                                 lhsT=onesr[:],
                                                 rhs=br2e[:, 0:512],
                                                 start=True, stop=False)
                            nc.tensor.matmul(out=pw0[:, tl2, :], lhsT=lhs,
                                             rhs=w2e[:, 0:512],
                                             start=not use_bias, stop=True)
                        # stage the pair to SBUF bf16 (ACT) so the products
                        # run in the fast all-SBUF 16-bit DVE mode
                        pwS = mp.tile([128, 2, 512], BF16, tag="pwS")
                        nc.scalar.copy(out=pwS[:], in_=pw0[:])
                        nc.vector.tensor_tensor(
                            out=P[:, sl2, 0:384].rearrange(
                                "p s (j i) -> p s j i", i=24),
                            in0=pwS[:, :, 0:384].rearrange(
                                "p s (j i) -> p s j i", i=24),
                            in1=g12[:, sl2, None, :].to_broadcast(
                                [128, 2, 16, 24]),
                            op=OP.mult)
                        nc.vector.tensor_tensor(
                            out=t3d[:, sl2],
                            in0=pwS[:, :, 384:512].rearrange(
                                "p s (j i) -> p s j i", i=16),
                            in1=st[:, sl2, None, :].to_broadcast(
                                [128, 2, 8, 16]),
                            op=OP.mult)
                    with nc.allow_low_precision(reason="16-term bf16 sum"):
                        nc.vector.reduce_sum(out=u3[:, sl4], in_=t3d[:, sl4],
                                             axis=mybir.AxisListType.X)
                    # stage p4 block to SBUF (ACT) so t4 runs in fast DVE mode
                    p4s = mp.tile([128, 4, 64], BF16, tag="p4s")
                    nc.scalar.copy(out=p4s[:], in_=pw1[:, :, 0:64])
                    g4r = g4[:, sl4, :].rearrange("p s (i c) -> p s i c", c=3)
                    for c in range(3):
                        nc.gpsimd.tensor_tensor(
                            out=t4d[:, sl4, c, :].rearrange(
                                "p s (j i) -> p s j i", i=8),
                            in0=p4s[:].rearrange("p s (j i) -> p s j i", i=8),
                            in1=g4r[:, :, :, c][:, :, None, :].to_broadcast(
                                [128, 4, 8, 8]),
                            op=OP.mult)
                    nc.gpsimd.tensor_tensor(
                        out=P[:, sl4, C_M13:C_M13 + 24].rearrange(
                            "p s (c j) -> p s c j", j=8),
                        in0=u3[:, sl4, None, :].to_broadcast([128, 4, 3, 8]),
                        in1=o3cs[:, sl4, 0:3, None].to_broadcast([128, 4, 3, 8]),
                        op=OP.mult)
                    with nc.allow_low_precision(reason="8-term bf16 sum"):
                        nc.vector.reduce_sum(
                            out=P[:, sl4, C_M14:C_M14 + 24],
                            in_=t4d[:, sl4].rearrange(
                                "p s c (j i) -> p s (c j) i", i=8),
                            axis=mybir.AxisListType.X)
                    # scatter this group's subtiles while the next group's
                    # matmuls proceed
                    for tl in range(4):
                        t = g * 4 + tl
                        nc.tensor.matmul(out=agg[:], lhsT=selt[:, t, :],
                                         rhs=P[:, t, :],
                                         start=(t == 0), stop=(t == SPS - 1))

                # node phase is software-pipelined by one supertile: emit the
                # PREVIOUS supertile's node phase here so it fills engine gaps
                # while this supertile's edge phase runs
                if pend is not None:
                    emit_node(*pend)
                pend = (s, agg, xo, inv)
            emit_node(*pend)

    nc.compile()
    return nc


_CACHE = {}


def kernel(**inputs):
    in_maps, metas, nsup, bg2, res, use_bias = _host_prep(**inputs)
    key = (nsup, bg2, res, use_bias)
    if key not in _CACHE:
        _CACHE[key] = build_program(nsup, bg2, res, use_bias)
    nc = _CACHE[key]
    r = run_bass_kernel_spmd(nc, in_maps, list(range(NCORE)))
    out = np.zeros((N, 40), np.float32)
    for k in range(NCORE):
        n0, n1, base_arr, span_arr, ns = metas[k]
        ob = r.results[k]["out"]
        for si in range(ns):
            sp = int(span_arr[si])
            if sp > 0:
                b = int(base_arr[si])
                out[b:b + sp] = ob[si, :sp]
    return out
